# revision 19
# baseline (speedup 1.0000x reference)
"""Trainium2 Bass kernel for nn_AttentionFold (self-contained).

Data-parallel over batch N=16: core i processes clouds {2i, 2i+1}.
Feature-major layout on-chip: activations stored [feature, point].

Key algebraic restructurings vs the reference:
  - glob (512-dim) contribution to gate/fold hidden layers is a per-cloud
    constant -> computed once per cloud as a matvec, applied as relu bias.
  - softmax bias sb2 folded into the filters: E = exp(z), F' = F * exp(sb2),
    sumexp = exp(sb2) . E  (softmax is shift/scale invariant in this form).
  - sigmoid(x) = 0.5 + 0.5*tanh(x/2) so the whole kernel uses one ACT
    table set (exp_and_others: exp/tanh/relu/identity/square).
  - coords grid is input-independent -> host constant, pre-transposed.
  - normalization scale 1/sqrt(max||c||^2) via DVE Newton rsqrt (no sqrt
    table set switch).
"""

import numpy as np

import concourse.bass as bass
import concourse.tile as tile
from concourse import bacc, mybir
from concourse import bass_utils

F32 = mybir.dt.float32
F32R = mybir.dt.float32r
U32 = mybir.dt.uint32
AF = mybir.ActivationFunctionType
ALU = mybir.AluOpType

N, P, C, HW, G = 16, 4096, 128, 784, 512
NCORES = 8
CPC = N // NCORES          # clouds per core = 2
NCH = P // 512             # chunks per cloud = 8
CH = 512                   # points per chunk
QT = [128, 128, 128, 128, 128, 128, 16]   # q tiles of HW=784
K_GRID = 64


def _build_program():
    """Build + compile the per-core Bass program once. Returns nc."""
    nc = bacc.Bacc("TRN2", target_bir_lowering=False, debug=False,
                   num_devices=NCORES)

    dt_in = {}

    def din(name, shape, dt=F32):
        dt_in[name] = nc.dram_tensor(name, shape, dt, kind="ExternalInput").ap()
        return dt_in[name]

    pts_d = din("pts", (CPC, P, 3))
    xf_d = din("xf", (CPC, P, 12))
    filt_d = din("filt", (CPC, C, HW))
    glob_d = din("glob", (CPC, G))
    w1aug_d = din("w1aug", (17, 384), F32R)
    sw2_d = din("sw2", (128, HW), F32R)
    expb_d = din("expb", (128, 7), F32R)
    expbf_d = din("expbf", (128, 7))
    gw1g_d = din("gw1g", (128, 512))
    fw1g_d = din("fw1g", (128, 512))
    fw1s_d = din("fw1s", (128, 128), F32R)
    gw2_d = din("gw2", (128, 128), F32R)
    gb2h_d = din("gb2h", (128, 1))
    fw2_d = din("fw2", (128, 128), F32R)
    fb2_d = din("fb2", (128, 1))
    fw3_d = din("fw3", (128, 3), F32R)
    fb3_d = din("fb3", (3, 1))
    sb1_d = din("sb1", (128, 1))
    gb1_d = din("gb1", (128, 1))
    fb1_d = din("fb1", (128, 1))
    coordsT_d = din("coordsT", (2, P), F32R)
    ident_d = din("ident", (128, 128))
    ident3_d = din("ident3", (3, 3), F32R)
    bc05_d = din("bc05", (1, 128), F32R)
    ones13_d = din("ones13", (1, 3))
    ones3_d = din("ones3", (3, 1))

    out_d = nc.dram_tensor("out", (CPC, P, 3), F32, kind="ExternalOutput").ap()

    with tile.TileContext(nc) as tc:
        from contextlib import ExitStack
        with ExitStack() as ctx:
            cpool = ctx.enter_context(tc.tile_pool(name="consts", bufs=1))
            clpool = ctx.enter_context(tc.tile_pool(name="cloud", bufs=2))
            cl1pool = ctx.enter_context(tc.tile_pool(name="cloud1", bufs=1))
            xpool = ctx.enter_context(tc.tile_pool(name="x", bufs=3))
            spool = ctx.enter_context(tc.tile_pool(name="acts", bufs=2))
            epool = ctx.enter_context(tc.tile_pool(name="e", bufs=2))
            # PSUM budget: 8 banks = z(2 banks x 2 bufs) + w(1 bank x 3) + sm(1)
            pz = ctx.enter_context(tc.tile_pool(name="pz", bufs=2, space="PSUM"))
            pw = ctx.enter_context(tc.tile_pool(name="pw", bufs=3, space="PSUM"))
            psm = ctx.enter_context(tc.tile_pool(name="psm", bufs=1, space="PSUM"))

            def cload(name, dram, shape, dt=F32):
                t = cpool.tile(shape, dt, tag=name)
                nc.sync.dma_start(t[:], dram[:])
                return t

            w1aug = cload("w1aug", w1aug_d, [17, 384], F32R)
            sw2 = cload("sw2", sw2_d, [128, HW], F32R)
            expb = cload("expb", expb_d, [128, 7], F32R)
            expbf = cload("expbf", expbf_d, [128, 7])
            gw1g = cload("gw1g", gw1g_d, [128, 512])
            fw1g = cload("fw1g", fw1g_d, [128, 512])
            fw1s = cload("fw1s", fw1s_d, [128, 128], F32R)
            gw2 = cload("gw2", gw2_d, [128, 128], F32R)
            gb2h = cload("gb2h", gb2h_d, [128, 1])
            fw2 = cload("fw2", fw2_d, [128, 128], F32R)
            fb2 = cload("fb2", fb2_d, [128, 1])
            fw3 = cload("fw3", fw3_d, [128, 3], F32R)
            fb3 = cload("fb3", fb3_d, [3, 1])
            sb1 = cload("sb1", sb1_d, [128, 1])
            gb1 = cload("gb1", gb1_d, [128, 1])
            fb1 = cload("fb1", fb1_d, [128, 1])
            ident = cload("ident", ident_d, [128, 128])
            ident3 = cload("ident3", ident3_d, [3, 3], F32R)
            bc05 = cload("bc05", bc05_d, [1, 128], F32R)
            ones13 = cload("ones13", ones13_d, [1, 3])
            ones3 = cload("ones3", ones3_d, [3, 1])
            rsqC = cpool.tile([1, 1], U32, tag="rsqC")
            nc.vector.memset(rsqC[:], 0x5F3759DF)

            for n in range(CPC):
                # ---- per-cloud prep ----
                Fsb = clpool.tile([128, HW], F32, tag="Fsb")
                nc.sync.dma_start(Fsb[:], filt_d[n])
                glob_sb = clpool.tile([128, 4], F32, tag="glob")
                nc.sync.dma_start(
                    glob_sb[:], glob_d[n].rearrange("(c p) -> p c", p=128))

                # F' = (F * exp(sb2)) transposed -> FT7 [q, c] tiles
                FT7 = clpool.tile([128, 896], F32R, tag="FT7")
                for j in range(7):
                    q = QT[j]
                    ftp = pw.tile([128, 128], F32, tag="w")
                    nc.tensor.transpose(
                        ftp[0:q, :], Fsb[:, 128 * j:128 * j + q], ident[:])
                    nc.vector.tensor_scalar_mul(
                        FT7[0:q, 128 * j:128 * (j + 1)], ftp[0:q, :],
                        expbf[0:q, j:j + 1])

                # glob matvecs -> per-cloud gate/fold biases
                gps = pw.tile([128, 1], F32, tag="w")
                for j in range(4):
                    nc.tensor.matmul(
                        gps[:], gw1g[:, 128 * j:128 * (j + 1)],
                        glob_sb[:, j:j + 1], start=(j == 0), stop=(j == 3))
                gbias = clpool.tile([128, 1], F32, tag="gbias")
                nc.vector.tensor_tensor(gbias[:], gps[:], gb1[:], ALU.add)
                fps = pw.tile([128, 1], F32, tag="w")
                for j in range(4):
                    nc.tensor.matmul(
                        fps[:], fw1g[:, 128 * j:128 * (j + 1)],
                        glob_sb[:, j:j + 1], start=(j == 0), stop=(j == 3))
                fbias = clpool.tile([128, 1], F32, tag="fbias")
                nc.vector.tensor_tensor(fbias[:], fps[:], fb1[:], ALU.add)

                opre = clpool.tile([3, P], F32, tag="opre")
                msum = clpool.tile([3, NCH], F32, tag="msum")

                # point-major staging for pts+transform (few, contiguous-run
                # descriptors); transposed to feature-major on the PE
                Xpm = clpool.tile([128, 480], F32, tag="Xpm")
                Xpm3 = Xpm[:, :].rearrange("p (b k) -> p b k", k=15)
                nc.gpsimd.dma_start(
                    Xpm3[:, :, 0:3],
                    pts_d[n].rearrange("(b p) k -> p b k", p=128))
                nc.gpsimd.dma_start(
                    Xpm3[:, :, 3:15],
                    xf_d[n].rearrange("(b p) k -> p b k", p=128))

                # ---- per-chunk pipeline ----
                for c in range(NCH):
                    sl = slice(CH * c, CH * (c + 1))
                    X17 = xpool.tile([17, CH], F32R, tag="X17")
                    XT = pw.tile([15, CH], F32, tag="w")
                    for b in range(4):
                        nc.tensor.transpose(
                            XT[:, 128 * b:128 * (b + 1)],
                            Xpm[:, 15 * (4 * c + b):15 * (4 * c + b) + 15],
                            ident[:])
                    nc.scalar.copy(X17[0:15, :], XT[:])
                    nc.sync.dma_start(X17[15:17, :], coordsT_d[:, sl])

                    sh_ps = pw.tile([128, CH], F32, tag="w")
                    nc.tensor.matmul(sh_ps[:], w1aug[:, 0:128],
                                     X17[:], start=True, stop=True)
                    gh_ps = pw.tile([128, CH], F32, tag="w")
                    nc.tensor.matmul(gh_ps[:], w1aug[:, 128:256],
                                     X17[:], start=True, stop=True)
                    sh = spool.tile([128, CH], F32R, tag="sh")
                    nc.vector.tensor_scalar(sh[:], sh_ps[:], sb1[:], 0.0,
                                            ALU.add, ALU.max)
                    gh = spool.tile([128, CH], F32R, tag="gh")
                    nc.vector.tensor_scalar(gh[:], gh_ps[:], gbias[:],
                                            0.0, ALU.add, ALU.max)

                    E = epool.tile([128, 3584], F32R, tag="E")
                    for r, js in enumerate([(0, 1), (2, 3), (4, 5), (6,)]):
                        zt = pz.tile([128, 1024], F32, tag="z")
                        for i, j in enumerate(js):
                            q = QT[j]
                            nc.tensor.matmul(
                                zt[0:q, 512 * i:512 * i + 512],
                                sw2[:, 128 * j:128 * j + q], sh[:],
                                start=True, stop=True)
                        w = 512 * len(js)
                        nc.scalar.activation(
                            E[:, 1024 * r:1024 * r + w], zt[:, 0:w], AF.Exp)

                    spat = pw.tile([128, CH], F32, tag="w")
                    sume = psm.tile([1, CH], F32, tag="sm")
                    for j in range(7):
                        q = QT[j]
                        esl = E[0:q, 512 * j:512 * (j + 1)]
                        nc.tensor.matmul(
                            spat[:], FT7[0:q, 128 * j:128 * (j + 1)],
                            esl, start=(j == 0), stop=(j == 6))
                        nc.tensor.matmul(
                            sume[:], expb[0:q, j:j + 1], esl,
                            start=(j == 0), stop=(j == 6))

                    ga = pw.tile([128, CH], F32, tag="w")
                    nc.tensor.matmul(ga[:], gw2[:], gh[:],
                                     start=True, stop=True)
                    gt = spool.tile([128, CH], F32, tag="gt")
                    nc.scalar.activation(gt[:], ga[:], AF.Tanh,
                                         bias=gb2h[:], scale=0.5)

                    rinv = spool.tile([1, CH], F32R, tag="rinv")
                    with nc.allow_low_precision(reason="fp32r rounding only"):
                        nc.vector.reciprocal(rinv[:], sume[:])
                    rbc = pw.tile([128, CH], F32, tag="w")
                    nc.tensor.matmul(rbc[:], bc05[:], rinv[:],
                                     start=True, stop=True)

                    g1 = spool.tile([128, CH], F32, tag="g1")
                    nc.vector.scalar_tensor_tensor(
                        g1[:], gt[:], 1.0, spat[:], ALU.add, ALU.mult)
                    feats = spool.tile([128, CH], F32R, tag="feats")
                    nc.vector.tensor_tensor(feats[:], g1[:], rbc[:], ALU.mult)

                    f1ps = pw.tile([128, CH], F32, tag="w")
                    nc.tensor.matmul(f1ps[:], w1aug[:, 256:384], X17[:],
                                     start=True, stop=False)
                    nc.tensor.matmul(f1ps[:], fw1s[:], feats[:],
                                     start=False, stop=True)
                    f1 = spool.tile([128, CH], F32R, tag="f1s")
                    nc.vector.tensor_scalar(f1[:], f1ps[:], fbias[:], 0.0,
                                            ALU.add, ALU.max)
                    f2ps = pw.tile([128, CH], F32, tag="w")
                    nc.tensor.matmul(f2ps[:], fw2[:], f1[:],
                                     start=True, stop=True)
                    f2 = spool.tile([128, CH], F32R, tag="f2s")
                    nc.vector.tensor_scalar(f2[:], f2ps[:], fb2[:], 0.0,
                                            ALU.add, ALU.max)
                    f3ps = psm.tile([3, CH], F32, tag="sm")
                    nc.tensor.matmul(f3ps[:], fw3[:], f2[:],
                                     start=True, stop=False)
                    nc.tensor.matmul(f3ps[:], ident3[:], X17[0:3, :],
                                     start=False, stop=True)
                    nc.scalar.activation(opre[:, sl], f3ps[:], AF.Identity,
                                         bias=fb3[:],
                                         accum_out=msum[:, c:c + 1])

                # ---- per-cloud normalize ----
                msr = spool.tile([3, 1], F32, tag="msr")
                nc.vector.reduce_sum(msr[:], msum[:], axis=mybir.AxisListType.X)
                negmean = spool.tile([3, 1], F32, tag="negmean")
                nc.vector.tensor_scalar_mul(negmean[:], msr[:], -1.0 / P)
                sqc = cl1pool.tile([3, P], F32, tag="sqc")
                nc.scalar.activation(sqc[:], opre[:], AF.Square,
                                     bias=negmean[:], scale=1.0)
                n2 = psm.tile([128, 32], F32, tag="sm")
                for c in range(NCH):
                    for b in range(4):
                        nc.tensor.matmul(
                            n2[:, 4 * c + b:4 * c + b + 1],
                            sqc[:, 512 * c + 128 * b:512 * c + 128 * (b + 1)],
                            ones3[:], start=True, stop=True)
                nm128 = spool.tile([128, 1], F32, tag="nm128")
                nc.vector.reduce_max(nm128[:], n2[:], axis=mybir.AxisListType.X)
                nmT = psm.tile([1, 128], F32, tag="sm")
                nc.tensor.transpose(nmT[:], nm128[:], ident[:])
                nmax = spool.tile([1, 1], F32, tag="nmax")
                nc.vector.reduce_max(nmax[:], nmT[:], axis=mybir.AxisListType.X)

                # Newton rsqrt: y ~= 1/sqrt(nmax)
                ysh = spool.tile([1, 1], U32, tag="ysh")
                nc.vector.tensor_scalar(ysh[:], nmax[:].bitcast(U32), 1, None,
                                        ALU.logical_shift_right)
                y = spool.tile([1, 1], F32, tag="y")
                nc.vector.tensor_tensor(y[:].bitcast(U32), rsqC[:], ysh[:],
                                        ALU.subtract)
                t = spool.tile([1, 1], F32, tag="t")
                for _ in range(4):
                    nc.vector.tensor_tensor(t[:], y[:], y[:], ALU.mult)
                    nc.vector.tensor_tensor(t[:], t[:], nmax[:], ALU.mult)
                    nc.vector.tensor_scalar(t[:], t[:], -0.5, 1.5,
                                            ALU.mult, ALU.add)
                    nc.vector.tensor_tensor(y[:], y[:], t[:], ALU.mult)

                rcolps = psm.tile([3, 1], F32, tag="sm")
                nc.tensor.matmul(rcolps[:], ones13[:], y[:],
                                 start=True, stop=True)
                rcol = spool.tile([3, 1], F32, tag="rcol")
                nc.vector.tensor_copy(rcol[:], rcolps[:])
                negmr = spool.tile([3, 1], F32, tag="negmr")
                nc.vector.tensor_tensor(negmr[:], negmean[:], rcol[:], ALU.mult)
                fin = cl1pool.tile([3, P], F32, tag="fin")
                nc.scalar.activation(fin[:], opre[:], AF.Identity,
                                     bias=negmr[:], scale=rcol[:])
                nc.sync.dma_start(out_d[n].rearrange("p c -> c p"), fin[:])

    nc.compile()
    return nc


_prog = None


def _get_prog():
    global _prog
    if _prog is None:
        _prog = _build_program()
    return _prog


def _spatial_grid(k):
    xs = np.linspace(-1.0, 1.0, k, dtype=np.float32)
    gx, gy = np.meshgrid(xs, xs, indexing="ij")
    return np.stack([gx.ravel(), gy.ravel()], axis=-1)


def _host_prep(inputs):
    """Per-call host constants derived from the (input) weights."""
    f32 = np.float32
    sw1 = np.asarray(inputs["sw1"], f32)
    sb1 = np.asarray(inputs["sb1"], f32)
    sw2 = np.asarray(inputs["sw2"], f32)
    sb2 = np.asarray(inputs["sb2"], f32)
    gw1 = np.asarray(inputs["gw1"], f32)
    gb1 = np.asarray(inputs["gb1"], f32)
    gw2 = np.asarray(inputs["gw2"], f32)
    gb2 = np.asarray(inputs["gb2"], f32)
    fw1 = np.asarray(inputs["fw1"], f32)
    fb1 = np.asarray(inputs["fb1"], f32)
    fw2 = np.asarray(inputs["fw2"], f32)
    fb2 = np.asarray(inputs["fb2"], f32)
    fw3 = np.asarray(inputs["fw3"], f32)
    fb3 = np.asarray(inputs["fb3"], f32)

    w1aug = np.zeros((17, 384), f32)
    w1aug[0:15, 0:128] = sw1
    w1aug[0:15, 128:256] = gw1[0:15]
    w1aug[0:15, 256:384] = fw1[0:15]
    w1aug[15:17, 256:384] = fw1[527:529]

    expb = np.zeros((128, 7), f32)
    eb = np.exp(sb2).astype(f32)
    for j in range(7):
        q = QT[j]
        expb[0:q, j] = eb[128 * j:128 * j + q]

    gw1g = np.concatenate(
        [gw1[15 + 128 * j:15 + 128 * (j + 1)] for j in range(4)], axis=1)
    fw1g = np.concatenate(
        [fw1[15 + 128 * j:15 + 128 * (j + 1)] for j in range(4)], axis=1)

    coordsT = np.ascontiguousarray(_spatial_grid(K_GRID).T)

    consts = {
        "w1aug": w1aug,
        "sw2": np.ascontiguousarray(sw2),
        "expb": expb,
        "expbf": expb,
        "gw1g": np.ascontiguousarray(gw1g),
        "fw1g": np.ascontiguousarray(fw1g),
        "fw1s": np.ascontiguousarray(fw1[529:657]),
        "gw2": np.ascontiguousarray(gw2),
        "gb2h": (0.5 * gb2).reshape(128, 1),
        "fw2": np.ascontiguousarray(fw2),
        "fb2": fb2.reshape(128, 1),
        "fw3": np.ascontiguousarray(fw3),
        "fb3": fb3.reshape(3, 1),
        "sb1": sb1.reshape(128, 1),
        "gb1": gb1.reshape(128, 1),
        "fb1": fb1.reshape(128, 1),
        "coordsT": coordsT,
        "ident": np.eye(128, dtype=f32),
        "ident3": np.eye(3, dtype=f32),
        "bc05": np.full((1, 128), 0.5, f32),
        "ones13": np.ones((1, 3), f32),
        "ones3": np.ones((3, 1), f32),
    }
    return {k: np.ascontiguousarray(v, f32) for k, v in consts.items()}


def _in_maps(inputs):
    f32 = np.float32
    pts = np.asarray(inputs["points"], f32)
    xf = np.asarray(inputs["transform"], f32)
    filt = np.asarray(inputs["enc_filters"], f32).reshape(N, C, HW)
    glob = np.asarray(inputs["enc_glob"], f32)
    consts = _host_prep(inputs)
    maps = []
    for i in range(NCORES):
        s = slice(CPC * i, CPC * (i + 1))
        m = {
            "pts": np.ascontiguousarray(pts[s]),
            "xf": np.ascontiguousarray(xf[s]),
            "filt": np.ascontiguousarray(filt[s]),
            "glob": np.ascontiguousarray(glob[s]),
        }
        m.update(consts)
        maps.append(m)
    return maps


def run(inputs, trace=False):
    nc = _get_prog()
    maps = _in_maps(inputs)
    res = bass_utils.run_bass_kernel_spmd(
        nc, maps, core_ids=list(range(NCORES)), trace=trace)
    out = np.concatenate([res.results[i]["out"] for i in range(NCORES)],
                         axis=0)
    return out.astype(np.float32), res


def kernel(**inputs):
    out, _ = run(inputs, trace=False)
    return out


# revision 20
# speedup vs baseline: 1.0517x; 1.0517x over previous
"""Trainium2 Bass kernel for nn_AttentionFold (self-contained).

Data-parallel over batch N=16: core i processes clouds {2i, 2i+1}.
Feature-major layout on-chip: activations stored [feature, point].

Key algebraic restructurings vs the reference:
  - glob (512-dim) contribution to gate/fold hidden layers is a per-cloud
    constant -> computed once per cloud as a matvec, applied as relu bias.
  - softmax bias sb2 folded into the filters: E = exp(z), F' = F * exp(sb2),
    sumexp = exp(sb2) . E  (softmax is shift/scale invariant in this form).
  - sigmoid(x) = 0.5 + 0.5*tanh(x/2) so the whole kernel uses one ACT
    table set (exp_and_others: exp/tanh/relu/identity/square).
  - coords grid is input-independent -> host constant, pre-transposed.
  - normalization scale 1/sqrt(max||c||^2) via DVE Newton rsqrt (no sqrt
    table set switch).
"""

import numpy as np

import concourse.bass as bass
import concourse.tile as tile
from concourse import bacc, mybir
from concourse import bass_utils

F32 = mybir.dt.float32
F32R = mybir.dt.float32r
U32 = mybir.dt.uint32
AF = mybir.ActivationFunctionType
ALU = mybir.AluOpType

N, P, C, HW, G = 16, 4096, 128, 784, 512
NCORES = 8
CPC = N // NCORES          # clouds per core = 2
NCH = P // 512             # chunks per cloud = 8
CH = 512                   # points per chunk
QT = [128, 128, 128, 128, 128, 128, 16]   # q tiles of HW=784
K_GRID = 64


def _build_program():
    """Build + compile the per-core Bass program once. Returns nc."""
    nc = bacc.Bacc("TRN2", target_bir_lowering=False, debug=False,
                   num_devices=NCORES)

    dt_in = {}

    def din(name, shape, dt=F32):
        dt_in[name] = nc.dram_tensor(name, shape, dt, kind="ExternalInput").ap()
        return dt_in[name]

    pts_d = din("pts", (CPC, P, 3))
    xf_d = din("xf", (CPC, P, 12))
    filt_d = din("filt", (CPC, C, HW))
    glob_d = din("glob", (CPC, G))
    w1aug_d = din("w1aug", (17, 384), F32R)
    sw2_d = din("sw2", (128, HW), F32R)
    expb_d = din("expb", (128, 7), F32R)
    expbf_d = din("expbf", (128, 7))
    gw1g_d = din("gw1g", (128, 512))
    fw1g_d = din("fw1g", (128, 512))
    fw1s_d = din("fw1s", (128, 128), F32R)
    gw2_d = din("gw2", (128, 128), F32R)
    gb2h_d = din("gb2h", (128, 1))
    fw2_d = din("fw2", (128, 128), F32R)
    fb2_d = din("fb2", (128, 1))
    fw3_d = din("fw3", (128, 3), F32R)
    fb3_d = din("fb3", (3, 1))
    sb1_d = din("sb1", (128, 1))
    gb1_d = din("gb1", (128, 1))
    fb1_d = din("fb1", (128, 1))
    coordsT_d = din("coordsT", (2, P), F32R)
    ident_d = din("ident", (128, 128))
    ident3_d = din("ident3", (3, 3), F32R)
    bc05_d = din("bc05", (1, 128), F32R)
    ones13_d = din("ones13", (1, 3))
    ones3_d = din("ones3", (3, 1))

    out_d = nc.dram_tensor("out", (CPC, P, 3), F32, kind="ExternalOutput").ap()

    with tile.TileContext(nc) as tc:
        from contextlib import ExitStack
        with ExitStack() as ctx:
            cpool = ctx.enter_context(tc.tile_pool(name="consts", bufs=1))
            clpool = ctx.enter_context(tc.tile_pool(name="cloud", bufs=2))
            cl1pool = ctx.enter_context(tc.tile_pool(name="cloud1", bufs=1))
            xpool = ctx.enter_context(tc.tile_pool(name="x", bufs=3))
            spool = ctx.enter_context(tc.tile_pool(name="acts", bufs=2))
            epool = ctx.enter_context(tc.tile_pool(name="e", bufs=2))
            # PSUM budget: 8 banks = z(1 bank x 2 bufs) + w(1 bank x 5) + sm(1)
            pz = ctx.enter_context(tc.tile_pool(name="pz", bufs=2, space="PSUM"))
            pw = ctx.enter_context(tc.tile_pool(name="pw", bufs=5, space="PSUM"))
            psm = ctx.enter_context(tc.tile_pool(name="psm", bufs=1, space="PSUM"))

            def cload(name, dram, shape, dt=F32):
                t = cpool.tile(shape, dt, tag=name)
                nc.sync.dma_start(t[:], dram[:])
                return t

            w1aug = cload("w1aug", w1aug_d, [17, 384], F32R)
            sw2 = cload("sw2", sw2_d, [128, HW], F32R)
            expb = cload("expb", expb_d, [128, 7], F32R)
            expbf = cload("expbf", expbf_d, [128, 7])
            gw1g = cload("gw1g", gw1g_d, [128, 512])
            fw1g = cload("fw1g", fw1g_d, [128, 512])
            fw1s = cload("fw1s", fw1s_d, [128, 128], F32R)
            gw2 = cload("gw2", gw2_d, [128, 128], F32R)
            gb2h = cload("gb2h", gb2h_d, [128, 1])
            fw2 = cload("fw2", fw2_d, [128, 128], F32R)
            fb2 = cload("fb2", fb2_d, [128, 1])
            fw3 = cload("fw3", fw3_d, [128, 3], F32R)
            fb3 = cload("fb3", fb3_d, [3, 1])
            sb1 = cload("sb1", sb1_d, [128, 1])
            gb1 = cload("gb1", gb1_d, [128, 1])
            fb1 = cload("fb1", fb1_d, [128, 1])
            ident = cload("ident", ident_d, [128, 128])
            ident3 = cload("ident3", ident3_d, [3, 3], F32R)
            bc05 = cload("bc05", bc05_d, [1, 128], F32R)
            ones13 = cload("ones13", ones13_d, [1, 3])
            ones3 = cload("ones3", ones3_d, [3, 1])
            rsqC = cpool.tile([1, 1], U32, tag="rsqC")
            nc.vector.memset(rsqC[:], 0x5F3759DF)

            for n in range(CPC):
                # ---- per-cloud prep ----
                Fsb = clpool.tile([128, HW], F32, tag="Fsb")
                nc.sync.dma_start(Fsb[:], filt_d[n])
                glob_sb = clpool.tile([128, 4], F32, tag="glob")
                nc.sync.dma_start(
                    glob_sb[:], glob_d[n].rearrange("(c p) -> p c", p=128))

                # F' = (F * exp(sb2)) transposed -> FT7 [q, c] tiles
                FT7 = clpool.tile([128, 896], F32R, tag="FT7")
                for j in range(7):
                    q = QT[j]
                    ftp = pw.tile([128, 128], F32, tag="w")
                    nc.tensor.transpose(
                        ftp[0:q, :], Fsb[:, 128 * j:128 * j + q], ident[:])
                    nc.vector.tensor_scalar_mul(
                        FT7[0:q, 128 * j:128 * (j + 1)], ftp[0:q, :],
                        expbf[0:q, j:j + 1])

                # glob matvecs -> per-cloud gate/fold biases
                gps = pw.tile([128, 1], F32, tag="w")
                for j in range(4):
                    nc.tensor.matmul(
                        gps[:], gw1g[:, 128 * j:128 * (j + 1)],
                        glob_sb[:, j:j + 1], start=(j == 0), stop=(j == 3))
                gbias = clpool.tile([128, 1], F32, tag="gbias")
                nc.vector.tensor_tensor(gbias[:], gps[:], gb1[:], ALU.add)
                fps = pw.tile([128, 1], F32, tag="w")
                for j in range(4):
                    nc.tensor.matmul(
                        fps[:], fw1g[:, 128 * j:128 * (j + 1)],
                        glob_sb[:, j:j + 1], start=(j == 0), stop=(j == 3))
                fbias = clpool.tile([128, 1], F32, tag="fbias")
                nc.vector.tensor_tensor(fbias[:], fps[:], fb1[:], ALU.add)

                opre = clpool.tile([3, P], F32, tag="opre")
                msum = clpool.tile([3, NCH], F32, tag="msum")

                # point-major staging for pts+transform (few, contiguous-run
                # descriptors); transposed to feature-major on the PE
                Xpm = clpool.tile([128, 480], F32, tag="Xpm")
                Xpm3 = Xpm[:, :].rearrange("p (b k) -> p b k", k=15)
                nc.gpsimd.dma_start(
                    Xpm3[:, :, 0:3],
                    pts_d[n].rearrange("(b p) k -> p b k", p=128))
                nc.gpsimd.dma_start(
                    Xpm3[:, :, 3:15],
                    xf_d[n].rearrange("(b p) k -> p b k", p=128))

                # ---- per-chunk pipeline ----
                for c in range(NCH):
                    sl = slice(CH * c, CH * (c + 1))
                    X17 = xpool.tile([17, CH], F32R, tag="X17")
                    XT = pw.tile([15, CH], F32, tag="w")
                    for b in range(4):
                        nc.tensor.transpose(
                            XT[:, 128 * b:128 * (b + 1)],
                            Xpm[:, 15 * (4 * c + b):15 * (4 * c + b) + 15],
                            ident[:])
                    nc.vector.tensor_copy(X17[0:15, :], XT[:])
                    nc.sync.dma_start(X17[15:17, :], coordsT_d[:, sl])

                    sh_ps = pw.tile([128, CH], F32, tag="w")
                    nc.tensor.matmul(sh_ps[:], w1aug[:, 0:128],
                                     X17[:], start=True, stop=True)
                    gh_ps = pw.tile([128, CH], F32, tag="w")
                    nc.tensor.matmul(gh_ps[:], w1aug[:, 128:256],
                                     X17[:], start=True, stop=True)
                    sh = spool.tile([128, CH], F32R, tag="sh")
                    nc.vector.tensor_scalar(sh[:], sh_ps[:], sb1[:], 0.0,
                                            ALU.add, ALU.max)
                    gh = spool.tile([128, CH], F32R, tag="gh")
                    nc.vector.tensor_scalar(gh[:], gh_ps[:], gbias[:],
                                            0.0, ALU.add, ALU.max)

                    E = epool.tile([128, 3584], F32R, tag="E")
                    for j in range(7):
                        q = QT[j]
                        zt = pz.tile([128, 512], F32, tag="z")
                        nc.tensor.matmul(
                            zt[0:q, :], sw2[:, 128 * j:128 * j + q], sh[:],
                            start=True, stop=True)
                        nc.scalar.activation(
                            E[0:q, 512 * j:512 * (j + 1)], zt[0:q, :], AF.Exp)

                    spat = pw.tile([128, CH], F32, tag="w")
                    sume = psm.tile([1, CH], F32, tag="sm")
                    for j in range(7):
                        q = QT[j]
                        esl = E[0:q, 512 * j:512 * (j + 1)]
                        nc.tensor.matmul(
                            spat[:], FT7[0:q, 128 * j:128 * (j + 1)],
                            esl, start=(j == 0), stop=(j == 6))
                        nc.tensor.matmul(
                            sume[:], expb[0:q, j:j + 1], esl,
                            start=(j == 0), stop=(j == 6))

                    ga = pw.tile([128, CH], F32, tag="w")
                    nc.tensor.matmul(ga[:], gw2[:], gh[:],
                                     start=True, stop=True)
                    gt = spool.tile([128, CH], F32, tag="gt")
                    nc.scalar.activation(gt[:], ga[:], AF.Tanh,
                                         bias=gb2h[:], scale=0.5)

                    rinv = spool.tile([1, CH], F32R, tag="rinv")
                    with nc.allow_low_precision(reason="fp32r rounding only"):
                        nc.vector.reciprocal(rinv[:], sume[:])
                    rbc = pw.tile([128, CH], F32, tag="w")
                    nc.tensor.matmul(rbc[:], bc05[:], rinv[:],
                                     start=True, stop=True)

                    g1 = spool.tile([128, CH], F32, tag="g1")
                    nc.vector.scalar_tensor_tensor(
                        g1[:], gt[:], 1.0, spat[:], ALU.add, ALU.mult)
                    feats = spool.tile([128, CH], F32R, tag="feats")
                    nc.vector.tensor_tensor(feats[:], g1[:], rbc[:], ALU.mult)

                    f1ps = pw.tile([128, CH], F32, tag="w")
                    nc.tensor.matmul(f1ps[:], w1aug[:, 256:384], X17[:],
                                     start=True, stop=False)
                    nc.tensor.matmul(f1ps[:], fw1s[:], feats[:],
                                     start=False, stop=True)
                    f1 = spool.tile([128, CH], F32R, tag="f1s")
                    nc.vector.tensor_scalar(f1[:], f1ps[:], fbias[:], 0.0,
                                            ALU.add, ALU.max)
                    f2ps = pw.tile([128, CH], F32, tag="w")
                    nc.tensor.matmul(f2ps[:], fw2[:], f1[:],
                                     start=True, stop=True)
                    f2 = spool.tile([128, CH], F32R, tag="f2s")
                    nc.vector.tensor_scalar(f2[:], f2ps[:], fb2[:], 0.0,
                                            ALU.add, ALU.max)
                    f3ps = psm.tile([3, CH], F32, tag="sm")
                    nc.tensor.matmul(f3ps[:], fw3[:], f2[:],
                                     start=True, stop=False)
                    nc.tensor.matmul(f3ps[:], ident3[:], X17[0:3, :],
                                     start=False, stop=True)
                    nc.scalar.activation(opre[:, sl], f3ps[:], AF.Identity,
                                         bias=fb3[:],
                                         accum_out=msum[:, c:c + 1])

                # ---- per-cloud normalize ----
                msr = spool.tile([3, 1], F32, tag="msr")
                nc.vector.reduce_sum(msr[:], msum[:], axis=mybir.AxisListType.X)
                negmean = spool.tile([3, 1], F32, tag="negmean")
                nc.vector.tensor_scalar_mul(negmean[:], msr[:], -1.0 / P)
                sqc = cl1pool.tile([3, P], F32, tag="sqc")
                nc.scalar.activation(sqc[:], opre[:], AF.Square,
                                     bias=negmean[:], scale=1.0)
                n2 = psm.tile([128, 32], F32, tag="sm")
                for c in range(NCH):
                    for b in range(4):
                        nc.tensor.matmul(
                            n2[:, 4 * c + b:4 * c + b + 1],
                            sqc[:, 512 * c + 128 * b:512 * c + 128 * (b + 1)],
                            ones3[:], start=True, stop=True)
                nm128 = spool.tile([128, 1], F32, tag="nm128")
                nc.vector.reduce_max(nm128[:], n2[:], axis=mybir.AxisListType.X)
                nmT = psm.tile([1, 128], F32, tag="sm")
                nc.tensor.transpose(nmT[:], nm128[:], ident[:])
                nmax = spool.tile([1, 1], F32, tag="nmax")
                nc.vector.reduce_max(nmax[:], nmT[:], axis=mybir.AxisListType.X)

                # Newton rsqrt: y ~= 1/sqrt(nmax)
                ysh = spool.tile([1, 1], U32, tag="ysh")
                nc.vector.tensor_scalar(ysh[:], nmax[:].bitcast(U32), 1, None,
                                        ALU.logical_shift_right)
                y = spool.tile([1, 1], F32, tag="y")
                nc.vector.tensor_tensor(y[:].bitcast(U32), rsqC[:], ysh[:],
                                        ALU.subtract)
                t = spool.tile([1, 1], F32, tag="t")
                for _ in range(4):
                    nc.vector.tensor_tensor(t[:], y[:], y[:], ALU.mult)
                    nc.vector.tensor_tensor(t[:], t[:], nmax[:], ALU.mult)
                    nc.vector.tensor_scalar(t[:], t[:], -0.5, 1.5,
                                            ALU.mult, ALU.add)
                    nc.vector.tensor_tensor(y[:], y[:], t[:], ALU.mult)

                rcolps = psm.tile([3, 1], F32, tag="sm")
                nc.tensor.matmul(rcolps[:], ones13[:], y[:],
                                 start=True, stop=True)
                rcol = spool.tile([3, 1], F32, tag="rcol")
                nc.vector.tensor_copy(rcol[:], rcolps[:])
                negmr = spool.tile([3, 1], F32, tag="negmr")
                nc.vector.tensor_tensor(negmr[:], negmean[:], rcol[:], ALU.mult)
                fin = cl1pool.tile([3, P], F32, tag="fin")
                nc.scalar.activation(fin[:], opre[:], AF.Identity,
                                     bias=negmr[:], scale=rcol[:])
                nc.sync.dma_start(out_d[n].rearrange("p c -> c p"), fin[:])

    nc.compile()
    return nc


_prog = None


def _get_prog():
    global _prog
    if _prog is None:
        _prog = _build_program()
    return _prog


def _spatial_grid(k):
    xs = np.linspace(-1.0, 1.0, k, dtype=np.float32)
    gx, gy = np.meshgrid(xs, xs, indexing="ij")
    return np.stack([gx.ravel(), gy.ravel()], axis=-1)


def _host_prep(inputs):
    """Per-call host constants derived from the (input) weights."""
    f32 = np.float32
    sw1 = np.asarray(inputs["sw1"], f32)
    sb1 = np.asarray(inputs["sb1"], f32)
    sw2 = np.asarray(inputs["sw2"], f32)
    sb2 = np.asarray(inputs["sb2"], f32)
    gw1 = np.asarray(inputs["gw1"], f32)
    gb1 = np.asarray(inputs["gb1"], f32)
    gw2 = np.asarray(inputs["gw2"], f32)
    gb2 = np.asarray(inputs["gb2"], f32)
    fw1 = np.asarray(inputs["fw1"], f32)
    fb1 = np.asarray(inputs["fb1"], f32)
    fw2 = np.asarray(inputs["fw2"], f32)
    fb2 = np.asarray(inputs["fb2"], f32)
    fw3 = np.asarray(inputs["fw3"], f32)
    fb3 = np.asarray(inputs["fb3"], f32)

    w1aug = np.zeros((17, 384), f32)
    w1aug[0:15, 0:128] = sw1
    w1aug[0:15, 128:256] = gw1[0:15]
    w1aug[0:15, 256:384] = fw1[0:15]
    w1aug[15:17, 256:384] = fw1[527:529]

    expb = np.zeros((128, 7), f32)
    eb = np.exp(sb2).astype(f32)
    for j in range(7):
        q = QT[j]
        expb[0:q, j] = eb[128 * j:128 * j + q]

    gw1g = np.concatenate(
        [gw1[15 + 128 * j:15 + 128 * (j + 1)] for j in range(4)], axis=1)
    fw1g = np.concatenate(
        [fw1[15 + 128 * j:15 + 128 * (j + 1)] for j in range(4)], axis=1)

    coordsT = np.ascontiguousarray(_spatial_grid(K_GRID).T)

    consts = {
        "w1aug": w1aug,
        "sw2": np.ascontiguousarray(sw2),
        "expb": expb,
        "expbf": expb,
        "gw1g": np.ascontiguousarray(gw1g),
        "fw1g": np.ascontiguousarray(fw1g),
        "fw1s": np.ascontiguousarray(fw1[529:657]),
        "gw2": np.ascontiguousarray(gw2),
        "gb2h": (0.5 * gb2).reshape(128, 1),
        "fw2": np.ascontiguousarray(fw2),
        "fb2": fb2.reshape(128, 1),
        "fw3": np.ascontiguousarray(fw3),
        "fb3": fb3.reshape(3, 1),
        "sb1": sb1.reshape(128, 1),
        "gb1": gb1.reshape(128, 1),
        "fb1": fb1.reshape(128, 1),
        "coordsT": coordsT,
        "ident": np.eye(128, dtype=f32),
        "ident3": np.eye(3, dtype=f32),
        "bc05": np.full((1, 128), 0.5, f32),
        "ones13": np.ones((1, 3), f32),
        "ones3": np.ones((3, 1), f32),
    }
    return {k: np.ascontiguousarray(v, f32) for k, v in consts.items()}


def _in_maps(inputs):
    f32 = np.float32
    pts = np.asarray(inputs["points"], f32)
    xf = np.asarray(inputs["transform"], f32)
    filt = np.asarray(inputs["enc_filters"], f32).reshape(N, C, HW)
    glob = np.asarray(inputs["enc_glob"], f32)
    consts = _host_prep(inputs)
    maps = []
    for i in range(NCORES):
        s = slice(CPC * i, CPC * (i + 1))
        m = {
            "pts": np.ascontiguousarray(pts[s]),
            "xf": np.ascontiguousarray(xf[s]),
            "filt": np.ascontiguousarray(filt[s]),
            "glob": np.ascontiguousarray(glob[s]),
        }
        m.update(consts)
        maps.append(m)
    return maps


def run(inputs, trace=False):
    nc = _get_prog()
    maps = _in_maps(inputs)
    res = bass_utils.run_bass_kernel_spmd(
        nc, maps, core_ids=list(range(NCORES)), trace=trace)
    out = np.concatenate([res.results[i]["out"] for i in range(NCORES)],
                         axis=0)
    return out.astype(np.float32), res


def kernel(**inputs):
    out, _ = run(inputs, trace=False)
    return out


# revision 22
# speedup vs baseline: 1.0799x; 1.0268x over previous
"""Trainium2 Bass kernel for nn_AttentionFold (self-contained).

Data-parallel over batch N=16: core i processes clouds {2i, 2i+1}.
Feature-major layout on-chip: activations stored [feature, point].

Key restructurings vs the reference:
  - glob (512-dim) contribution to gate/fold hidden layers is a per-cloud
    constant -> one matvec per cloud, applied as relu bias.
  - softmax bias sb2 folded into the filters: E = exp(z), F' = F * exp(sb2),
    sumexp = exp(sb2) . E  (softmax invariant under this refactoring).
  - sigmoid(x) = 0.5 + 0.5*tanh(x/2) so one ACT table set serves the whole
    kernel (exp_and_others: exp/tanh/relu/identity/square).
  - coords grid is input-independent -> host constant, pre-transposed.
  - normalization scale 1/sqrt(max||c||^2) via DVE Newton rsqrt.
  - matmuls in bf16 (fp32 PSUM accumulate) so weight loads use FWL and
    hide behind the matmul stream; the points passthrough into the output
    stays fp32 for accuracy.
"""

from contextlib import ExitStack

import numpy as np
import ml_dtypes

import concourse.bass as bass
import concourse.tile as tile
from concourse import bacc, mybir
from concourse import bass_utils

F32 = mybir.dt.float32
BF16 = mybir.dt.bfloat16
U32 = mybir.dt.uint32
AF = mybir.ActivationFunctionType
ALU = mybir.AluOpType
BF = ml_dtypes.bfloat16

N, P, C, HW, G = 16, 4096, 128, 784, 512
NCORES = 8
CPC = N // NCORES          # clouds per core = 2
CH = 512                   # points per chunk
NPAIR = P // (2 * CH)      # chunk pairs per cloud = 4
QT = [128, 128, 128, 128, 128, 128, 16]   # q tiles of HW=784
K_GRID = 64


def _build_program():
    nc = bacc.Bacc("TRN2", target_bir_lowering=False, debug=False,
                   num_devices=NCORES)

    def din(name, shape, dt=F32):
        return nc.dram_tensor(name, shape, dt, kind="ExternalInput").ap()

    pts_d = din("pts", (CPC, P, 3))
    xf_d = din("xf", (CPC, P, 12))
    filt_d = din("filt", (CPC, C, HW))
    glob_d = din("glob", (CPC, G))
    w1aug_d = din("w1aug", (17, 384), BF16)
    sw2_d = din("sw2", (128, HW), BF16)
    expb_d = din("expb", (128, 7), BF16)
    expbf_d = din("expbf", (128, 7))
    gw1g_d = din("gw1g", (128, 512))
    fw1g_d = din("fw1g", (128, 512))
    fw1s_d = din("fw1s", (128, 128), BF16)
    gw2_d = din("gw2", (128, 128), BF16)
    gb2h_d = din("gb2h", (128, 1))
    fw2_d = din("fw2", (128, 128), BF16)
    fb2_d = din("fb2", (128, 1))
    fw3_d = din("fw3", (128, 3), BF16)
    fb3_d = din("fb3", (3, 1))
    sb1_d = din("sb1", (128, 1))
    gb1_d = din("gb1", (128, 1))
    fb1_d = din("fb1", (128, 1))
    coordsT_d = din("coordsT", (2, P), BF16)
    ident_d = din("ident", (128, 128))
    bc05_d = din("bc05", (1, 128), BF16)
    ones13_d = din("ones13", (1, 3))
    ones3_d = din("ones3", (3, 1))

    out_d = nc.dram_tensor("out", (CPC, P, 3), F32, kind="ExternalOutput").ap()

    with tile.TileContext(nc) as tc, ExitStack() as ctx:
        cpool = ctx.enter_context(tc.tile_pool(name="consts", bufs=1))
        clpool = ctx.enter_context(tc.tile_pool(name="cloud", bufs=2))
        cl1pool = ctx.enter_context(tc.tile_pool(name="cloud1", bufs=1))
        spool = ctx.enter_context(tc.tile_pool(name="acts", bufs=2))
        epool = ctx.enter_context(tc.tile_pool(name="e", bufs=2))
        # PSUM: 8 banks = z pair-wide (2 banks x 1 buf) + w (1 bank x 5) + sm
        pz = ctx.enter_context(tc.tile_pool(name="pz", bufs=1, space="PSUM"))
        pw = ctx.enter_context(tc.tile_pool(name="pw", bufs=5, space="PSUM"))
        psm = ctx.enter_context(tc.tile_pool(name="psm", bufs=1, space="PSUM"))

        def cload(name, dram, shape, dt=F32):
            t = cpool.tile(shape, dt, tag=name)
            nc.sync.dma_start(t[:], dram[:])
            return t

        w1aug = cload("w1aug", w1aug_d, [17, 384], BF16)
        sw2 = cload("sw2", sw2_d, [128, HW], BF16)
        expb = cload("expb", expb_d, [128, 7], BF16)
        expbf = cload("expbf", expbf_d, [128, 7])
        gw1g = cload("gw1g", gw1g_d, [128, 512])
        fw1g = cload("fw1g", fw1g_d, [128, 512])
        fw1s = cload("fw1s", fw1s_d, [128, 128], BF16)
        gw2 = cload("gw2", gw2_d, [128, 128], BF16)
        gb2h = cload("gb2h", gb2h_d, [128, 1])
        fw2 = cload("fw2", fw2_d, [128, 128], BF16)
        fb2 = cload("fb2", fb2_d, [128, 1])
        fw3 = cload("fw3", fw3_d, [128, 3], BF16)
        fb3 = cload("fb3", fb3_d, [3, 1])
        sb1 = cload("sb1", sb1_d, [128, 1])
        gb1 = cload("gb1", gb1_d, [128, 1])
        fb1 = cload("fb1", fb1_d, [128, 1])
        ident = cload("ident", ident_d, [128, 128])
        bc05 = cload("bc05", bc05_d, [1, 128], BF16)
        ones13 = cload("ones13", ones13_d, [1, 3])
        ones3 = cload("ones3", ones3_d, [3, 1])
        rsqC = cpool.tile([1, 1], U32, tag="rsqC")
        nc.vector.memset(rsqC[:], 0x5F3759DF)

        for n in range(CPC):
            # ---- per-cloud prep ----
            Fsb = clpool.tile([128, HW], F32, tag="Fsb")
            nc.sync.dma_start(Fsb[:], filt_d[n])
            glob_sb = clpool.tile([128, 4], F32, tag="glob")
            nc.sync.dma_start(
                glob_sb[:], glob_d[n].rearrange("(c p) -> p c", p=128))

            # F' = (F * exp(sb2)) transposed -> FT7 [q, c] tiles (bf16)
            FT7 = clpool.tile([128, 896], BF16, tag="FT7")
            for j in range(7):
                q = QT[j]
                ftp = pw.tile([128, 128], F32, tag="w")
                nc.tensor.transpose(
                    ftp[0:q, :], Fsb[:, 128 * j:128 * j + q], ident[:])
                nc.vector.tensor_scalar_mul(
                    FT7[0:q, 128 * j:128 * (j + 1)], ftp[0:q, :],
                    expbf[0:q, j:j + 1])

            # glob matvecs -> per-cloud gate/fold relu biases
            gps = pw.tile([128, 1], F32, tag="w")
            for j in range(4):
                nc.tensor.matmul(
                    gps[:], gw1g[:, 128 * j:128 * (j + 1)],
                    glob_sb[:, j:j + 1], start=(j == 0), stop=(j == 3))
            gbias = clpool.tile([128, 1], F32, tag="gbias")
            nc.vector.tensor_tensor(gbias[:], gps[:], gb1[:], ALU.add)
            fps = pw.tile([128, 1], F32, tag="w")
            for j in range(4):
                nc.tensor.matmul(
                    fps[:], fw1g[:, 128 * j:128 * (j + 1)],
                    glob_sb[:, j:j + 1], start=(j == 0), stop=(j == 3))
            fbias = clpool.tile([128, 1], F32, tag="fbias")
            nc.vector.tensor_tensor(fbias[:], fps[:], fb1[:], ALU.add)

            opre = clpool.tile([3, P], F32, tag="opre")
            msum = clpool.tile([3, 2 * NPAIR], F32, tag="msum")

            # point-major staging for pts+transform; transposed on the PE
            Xpm = clpool.tile([128, 480], F32, tag="Xpm")
            Xpm3 = Xpm[:, :].rearrange("p (b k) -> p b k", k=15)
            nc.gpsimd.dma_start(
                Xpm3[:, :, 0:3], pts_d[n].rearrange("(b p) k -> p b k", p=128))
            nc.gpsimd.dma_start(
                Xpm3[:, :, 3:15], xf_d[n].rearrange("(b p) k -> p b k", p=128))

            # ---- chunk pairs ----
            for cp in range(NPAIR):
                sh2 = spool.tile([128, 2 * CH], BF16, tag="sh2")
                ghs = []
                X17s = []
                pts32s = []
                for h in range(2):
                    c = 2 * cp + h
                    sl = slice(CH * c, CH * (c + 1))
                    X17 = spool.tile([17, CH], BF16, tag=f"X17_{h}")
                    XT = pw.tile([15, CH], F32, tag="w")
                    for b in range(4):
                        nc.tensor.transpose(
                            XT[:, 128 * b:128 * (b + 1)],
                            Xpm[:, 15 * (4 * c + b):15 * (4 * c + b) + 15],
                            ident[:])
                    nc.vector.tensor_copy(X17[0:15, :], XT[:])
                    pts32 = spool.tile([3, CH], F32, tag=f"pts32_{h}")
                    nc.scalar.copy(pts32[:], XT[0:3, :])
                    nc.sync.dma_start(X17[15:17, :], coordsT_d[:, sl])

                    sh_ps = pw.tile([128, CH], F32, tag="w")
                    nc.tensor.matmul(sh_ps[:], w1aug[:, 0:128], X17[:],
                                     start=True, stop=True)
                    gh_ps = pw.tile([128, CH], F32, tag="w")
                    nc.tensor.matmul(gh_ps[:], w1aug[:, 128:256], X17[:],
                                     start=True, stop=True)
                    nc.vector.tensor_scalar(
                        sh2[:, CH * h:CH * (h + 1)], sh_ps[:], sb1[:], 0.0,
                        ALU.add, ALU.max)
                    gh = spool.tile([128, CH], BF16, tag=f"gh_{h}")
                    nc.vector.tensor_scalar(gh[:], gh_ps[:], gbias[:],
                                            0.0, ALU.add, ALU.max)
                    ghs.append(gh)
                    X17s.append(X17)
                    pts32s.append(pts32)

                # z + exp, pair-wide (N=1024 bf16 moving operand)
                E = epool.tile([128, 7 * 2 * CH], BF16, tag="E")
                for j in range(7):
                    q = QT[j]
                    zt = pz.tile([128, 2 * CH], F32, tag="z")
                    for h in range(2):
                        nc.tensor.matmul(zt[0:q, CH * h:CH * (h + 1)],
                                         sw2[:, 128 * j:128 * j + q],
                                         sh2[:, CH * h:CH * (h + 1)],
                                         start=True, stop=True)
                    nc.scalar.activation(
                        E[0:q, 1024 * j:1024 * (j + 1)], zt[0:q, :], AF.Exp)

                for h in range(2):
                    c = 2 * cp + h
                    sl = slice(CH * c, CH * (c + 1))
                    X17, gh, pts32 = X17s[h], ghs[h], pts32s[h]

                    spat = pw.tile([128, CH], F32, tag="w")
                    sume = psm.tile([1, CH], F32, tag="sm")
                    for j in range(7):
                        q = QT[j]
                        esl = E[0:q, 1024 * j + CH * h:1024 * j + CH * (h + 1)]
                        nc.tensor.matmul(
                            spat[:], FT7[0:q, 128 * j:128 * (j + 1)], esl,
                            start=(j == 0), stop=(j == 6))
                        nc.tensor.matmul(
                            sume[:], expb[0:q, j:j + 1], esl,
                            start=(j == 0), stop=(j == 6))

                    ga = pw.tile([128, CH], F32, tag="w")
                    nc.tensor.matmul(ga[:], gw2[:], gh[:],
                                     start=True, stop=True)
                    gt = spool.tile([128, CH], F32, tag="gt")
                    nc.scalar.activation(gt[:], ga[:], AF.Tanh,
                                         bias=gb2h[:], scale=0.5)

                    rinv = spool.tile([1, CH], BF16, tag="rinv")
                    with nc.allow_low_precision(reason="bf16 softmax scale"):
                        nc.vector.reciprocal(rinv[:], sume[:])
                    rbc = pw.tile([128, CH], F32, tag="w")
                    nc.tensor.matmul(rbc[:], bc05[:], rinv[:],
                                     start=True, stop=True)

                    g1 = spool.tile([128, CH], F32, tag="g1")
                    nc.vector.scalar_tensor_tensor(
                        g1[:], gt[:], 1.0, spat[:], ALU.add, ALU.mult)
                    feats = spool.tile([128, CH], BF16, tag="feats")
                    nc.vector.tensor_tensor(feats[:], g1[:], rbc[:], ALU.mult)

                    f1ps = pw.tile([128, CH], F32, tag="w")
                    nc.tensor.matmul(f1ps[:], w1aug[:, 256:384], X17[:],
                                     start=True, stop=False)
                    nc.tensor.matmul(f1ps[:], fw1s[:], feats[:],
                                     start=False, stop=True)
                    f1 = spool.tile([128, CH], BF16, tag="f1s")
                    nc.vector.tensor_scalar(f1[:], f1ps[:], fbias[:], 0.0,
                                            ALU.add, ALU.max)
                    f2ps = pw.tile([128, CH], F32, tag="w")
                    nc.tensor.matmul(f2ps[:], fw2[:], f1[:],
                                     start=True, stop=True)
                    f2 = spool.tile([128, CH], BF16, tag="f2s")
                    nc.vector.tensor_scalar(f2[:], f2ps[:], fb2[:], 0.0,
                                            ALU.add, ALU.max)
                    f3ps = psm.tile([3, CH], F32, tag="sm")
                    nc.tensor.matmul(f3ps[:], fw3[:], f2[:],
                                     start=True, stop=True)
                    # opre = f3 + fb3 + points (fp32 path); accum -> mean
                    nc.vector.scalar_tensor_tensor(
                        opre[:, sl], f3ps[:], fb3[:], pts32[:],
                        ALU.add, ALU.add, accum_out=msum[:, c:c + 1])

            # ---- per-cloud normalize ----
            msr = spool.tile([3, 1], F32, tag="msr")
            nc.vector.reduce_sum(msr[:], msum[:], axis=mybir.AxisListType.X)
            negmean = spool.tile([3, 1], F32, tag="negmean")
            nc.vector.tensor_scalar_mul(negmean[:], msr[:], -1.0 / P)
            sqc = cl1pool.tile([3, P], F32, tag="sqc")
            nc.scalar.activation(sqc[:], opre[:], AF.Square,
                                 bias=negmean[:], scale=1.0)
            n2 = psm.tile([128, 32], F32, tag="sm")
            for c in range(2 * NPAIR):
                for b in range(4):
                    nc.tensor.matmul(
                        n2[:, 4 * c + b:4 * c + b + 1],
                        sqc[:, 512 * c + 128 * b:512 * c + 128 * (b + 1)],
                        ones3[:], start=True, stop=True)
            nm128 = spool.tile([128, 1], F32, tag="nm128")
            nc.vector.reduce_max(nm128[:], n2[:], axis=mybir.AxisListType.X)
            nmT = psm.tile([1, 128], F32, tag="sm")
            nc.tensor.transpose(nmT[:], nm128[:], ident[:])
            nmax = spool.tile([1, 1], F32, tag="nmax")
            nc.vector.reduce_max(nmax[:], nmT[:], axis=mybir.AxisListType.X)

            # Newton rsqrt: y ~= 1/sqrt(nmax)
            ysh = spool.tile([1, 1], U32, tag="ysh")
            nc.vector.tensor_scalar(ysh[:], nmax[:].bitcast(U32), 1, None,
                                    ALU.logical_shift_right)
            y = spool.tile([1, 1], F32, tag="y")
            nc.vector.tensor_tensor(y[:].bitcast(U32), rsqC[:], ysh[:],
                                    ALU.subtract)
            t = spool.tile([1, 1], F32, tag="t")
            for _ in range(4):
                nc.vector.tensor_tensor(t[:], y[:], y[:], ALU.mult)
                nc.vector.tensor_tensor(t[:], t[:], nmax[:], ALU.mult)
                nc.vector.tensor_scalar(t[:], t[:], -0.5, 1.5,
                                        ALU.mult, ALU.add)
                nc.vector.tensor_tensor(y[:], y[:], t[:], ALU.mult)

            rcolps = psm.tile([3, 1], F32, tag="sm")
            nc.tensor.matmul(rcolps[:], ones13[:], y[:],
                             start=True, stop=True)
            rcol = spool.tile([3, 1], F32, tag="rcol")
            nc.vector.tensor_copy(rcol[:], rcolps[:])
            negmr = spool.tile([3, 1], F32, tag="negmr")
            nc.vector.tensor_tensor(negmr[:], negmean[:], rcol[:], ALU.mult)
            fin = cl1pool.tile([3, P], F32, tag="fin")
            nc.scalar.activation(fin[:], opre[:], AF.Identity,
                                 bias=negmr[:], scale=rcol[:])
            nc.sync.dma_start(out_d[n].rearrange("p c -> c p"), fin[:])

    nc.compile()
    return nc


_prog = None


def _get_prog():
    global _prog
    if _prog is None:
        _prog = _build_program()
    return _prog


def _spatial_grid(k):
    xs = np.linspace(-1.0, 1.0, k, dtype=np.float32)
    gx, gy = np.meshgrid(xs, xs, indexing="ij")
    return np.stack([gx.ravel(), gy.ravel()], axis=-1)


def _host_prep(inputs):
    f32 = np.float32
    sw1 = np.asarray(inputs["sw1"], f32)
    sb1 = np.asarray(inputs["sb1"], f32)
    sw2 = np.asarray(inputs["sw2"], f32)
    sb2 = np.asarray(inputs["sb2"], f32)
    gw1 = np.asarray(inputs["gw1"], f32)
    gb1 = np.asarray(inputs["gb1"], f32)
    gw2 = np.asarray(inputs["gw2"], f32)
    gb2 = np.asarray(inputs["gb2"], f32)
    fw1 = np.asarray(inputs["fw1"], f32)
    fb1 = np.asarray(inputs["fb1"], f32)
    fw2 = np.asarray(inputs["fw2"], f32)
    fb2 = np.asarray(inputs["fb2"], f32)
    fw3 = np.asarray(inputs["fw3"], f32)
    fb3 = np.asarray(inputs["fb3"], f32)

    w1aug = np.zeros((17, 384), f32)
    w1aug[0:15, 0:128] = sw1
    w1aug[0:15, 128:256] = gw1[0:15]
    w1aug[0:15, 256:384] = fw1[0:15]
    w1aug[15:17, 256:384] = fw1[527:529]

    expb = np.zeros((128, 7), f32)
    eb = np.exp(sb2).astype(f32)
    for j in range(7):
        q = QT[j]
        expb[0:q, j] = eb[128 * j:128 * j + q]

    gw1g = np.concatenate(
        [gw1[15 + 128 * j:15 + 128 * (j + 1)] for j in range(4)], axis=1)
    fw1g = np.concatenate(
        [fw1[15 + 128 * j:15 + 128 * (j + 1)] for j in range(4)], axis=1)

    coordsT = np.ascontiguousarray(_spatial_grid(K_GRID).T)

    consts = {
        "w1aug": w1aug.astype(BF),
        "sw2": np.ascontiguousarray(sw2).astype(BF),
        "expb": expb.astype(BF),
        "expbf": expb,
        "gw1g": np.ascontiguousarray(gw1g),
        "fw1g": np.ascontiguousarray(fw1g),
        "fw1s": np.ascontiguousarray(fw1[529:657]).astype(BF),
        "gw2": np.ascontiguousarray(gw2).astype(BF),
        "gb2h": (0.5 * gb2).reshape(128, 1),
        "fw2": np.ascontiguousarray(fw2).astype(BF),
        "fb2": fb2.reshape(128, 1),
        "fw3": np.ascontiguousarray(fw3).astype(BF),
        "fb3": fb3.reshape(3, 1),
        "sb1": sb1.reshape(128, 1),
        "gb1": gb1.reshape(128, 1),
        "fb1": fb1.reshape(128, 1),
        "coordsT": coordsT.astype(BF),
        "ident": np.eye(128, dtype=f32),
        "bc05": np.full((1, 128), 0.5, f32).astype(BF),
        "ones13": np.ones((1, 3), f32),
        "ones3": np.ones((3, 1), f32),
    }
    return {k: (np.ascontiguousarray(v) if v.dtype == BF
                else np.ascontiguousarray(v, f32))
            for k, v in consts.items()}


def _in_maps(inputs):
    f32 = np.float32
    pts = np.asarray(inputs["points"], f32)
    xf = np.asarray(inputs["transform"], f32)
    filt = np.asarray(inputs["enc_filters"], f32).reshape(N, C, HW)
    glob = np.asarray(inputs["enc_glob"], f32)
    consts = _host_prep(inputs)
    maps = []
    for i in range(NCORES):
        s = slice(CPC * i, CPC * (i + 1))
        m = {
            "pts": np.ascontiguousarray(pts[s]),
            "xf": np.ascontiguousarray(xf[s]),
            "filt": np.ascontiguousarray(filt[s]),
            "glob": np.ascontiguousarray(glob[s]),
        }
        m.update(consts)
        maps.append(m)
    return maps


def run(inputs, trace=False):
    nc = _get_prog()
    maps = _in_maps(inputs)
    res = bass_utils.run_bass_kernel_spmd(
        nc, maps, core_ids=list(range(NCORES)), trace=trace)
    out = np.concatenate([res.results[i]["out"] for i in range(NCORES)],
                         axis=0)
    return out.astype(np.float32), res


def kernel(**inputs):
    out, _ = run(inputs, trace=False)
    return out


# revision 23
# speedup vs baseline: 1.1020x; 1.0204x over previous
"""Trainium2 Bass kernel for nn_AttentionFold (self-contained).

Data-parallel over batch N=16: core i processes clouds {2i, 2i+1}.
Feature-major layout on-chip: activations stored [feature, point].

Key restructurings vs the reference:
  - glob (512-dim) contribution to gate/fold hidden layers is a per-cloud
    constant -> one matvec per cloud, applied as relu bias.
  - softmax bias sb2 folded into the filters: E = exp(z), F' = F * exp(sb2),
    sumexp = exp(sb2) . E  (softmax invariant under this refactoring).
  - sigmoid(x) = 0.5 + 0.5*tanh(x/2) so one ACT table set serves the whole
    kernel (exp_and_others: exp/tanh/relu/identity/square).
  - coords grid is input-independent -> host constant, pre-transposed.
  - normalization scale 1/sqrt(max||c||^2) via DVE Newton rsqrt.
  - matmuls in bf16 (fp32 PSUM accumulate) so weight loads use FWL and
    hide behind the matmul stream; the points passthrough into the output
    stays fp32 for accuracy.
"""

from contextlib import ExitStack

import numpy as np
import ml_dtypes

import concourse.bass as bass
import concourse.tile as tile
from concourse import bacc, mybir
from concourse import bass_utils

F32 = mybir.dt.float32
BF16 = mybir.dt.bfloat16
U32 = mybir.dt.uint32
AF = mybir.ActivationFunctionType
ALU = mybir.AluOpType
BF = ml_dtypes.bfloat16

N, P, C, HW, G = 16, 4096, 128, 784, 512
NCORES = 8
CPC = N // NCORES          # clouds per core = 2
CH = 512                   # points per chunk
NPAIR = P // (2 * CH)      # chunk pairs per cloud = 4
QT = [128, 128, 128, 128, 128, 128, 16]   # q tiles of HW=784
K_GRID = 64


def _build_program():
    nc = bacc.Bacc("TRN2", target_bir_lowering=False, debug=False,
                   num_devices=NCORES)

    def din(name, shape, dt=F32):
        return nc.dram_tensor(name, shape, dt, kind="ExternalInput").ap()

    pts_d = din("pts", (CPC, P, 3))
    xf_d = din("xf", (CPC, P, 12))
    filt_d = din("filt", (CPC, C, HW))
    glob_d = din("glob", (CPC, G))
    w1aug_d = din("w1aug", (17, 384), BF16)
    sw2_d = din("sw2", (128, HW), BF16)
    expb_d = din("expb", (128, 7), BF16)
    expbf_d = din("expbf", (128, 7))
    gw1g_d = din("gw1g", (128, 512))
    fw1g_d = din("fw1g", (128, 512))
    fw1s_d = din("fw1s", (128, 128), BF16)
    gw2_d = din("gw2", (128, 128), BF16)
    gb2h_d = din("gb2h", (128, 1))
    fw2_d = din("fw2", (128, 128), BF16)
    fb2_d = din("fb2", (128, 1))
    fw3_d = din("fw3", (128, 3), BF16)
    fb3_d = din("fb3", (3, 1))
    sb1_d = din("sb1", (128, 1))
    gb1_d = din("gb1", (128, 1))
    fb1_d = din("fb1", (128, 1))
    coordsT_d = din("coordsT", (2, P), BF16)
    ident_d = din("ident", (128, 128))
    bc05_d = din("bc05", (1, 128), BF16)
    ones13_d = din("ones13", (1, 3))
    ones3_d = din("ones3", (3, 1))

    out_d = nc.dram_tensor("out", (CPC, P, 3), F32, kind="ExternalOutput").ap()

    with tile.TileContext(nc) as tc, ExitStack() as ctx:
        cpool = ctx.enter_context(tc.tile_pool(name="consts", bufs=1))
        clpool = ctx.enter_context(tc.tile_pool(name="cloud", bufs=2))
        cl1pool = ctx.enter_context(tc.tile_pool(name="cloud1", bufs=1))
        spool = ctx.enter_context(tc.tile_pool(name="acts", bufs=2))
        epool = ctx.enter_context(tc.tile_pool(name="e", bufs=2))
        # PSUM: 8 banks = z pair-wide (2 banks x 1 buf) + w (1 bank x 5) + sm
        pz = ctx.enter_context(tc.tile_pool(name="pz", bufs=1, space="PSUM"))
        pw = ctx.enter_context(tc.tile_pool(name="pw", bufs=4, space="PSUM"))
        psm = ctx.enter_context(tc.tile_pool(name="psm", bufs=2, space="PSUM"))

        def cload(name, dram, shape, dt=F32):
            t = cpool.tile(shape, dt, tag=name)
            nc.sync.dma_start(t[:], dram[:])
            return t

        w1aug = cload("w1aug", w1aug_d, [17, 384], BF16)
        sw2 = cload("sw2", sw2_d, [128, HW], BF16)
        expb = cload("expb", expb_d, [128, 7], BF16)
        expbf = cload("expbf", expbf_d, [128, 7])
        gw1g = cload("gw1g", gw1g_d, [128, 512])
        fw1g = cload("fw1g", fw1g_d, [128, 512])
        fw1s = cload("fw1s", fw1s_d, [128, 128], BF16)
        gw2 = cload("gw2", gw2_d, [128, 128], BF16)
        gb2h = cload("gb2h", gb2h_d, [128, 1])
        fw2 = cload("fw2", fw2_d, [128, 128], BF16)
        fb2 = cload("fb2", fb2_d, [128, 1])
        fw3 = cload("fw3", fw3_d, [128, 3], BF16)
        fb3 = cload("fb3", fb3_d, [3, 1])
        sb1 = cload("sb1", sb1_d, [128, 1])
        gb1 = cload("gb1", gb1_d, [128, 1])
        fb1 = cload("fb1", fb1_d, [128, 1])
        ident = cload("ident", ident_d, [128, 128])
        bc05 = cload("bc05", bc05_d, [1, 128], BF16)
        ones13 = cload("ones13", ones13_d, [1, 3])
        ones3 = cload("ones3", ones3_d, [3, 1])
        rsqC = cpool.tile([1, 1], U32, tag="rsqC")
        nc.vector.memset(rsqC[:], 0x5F3759DF)

        for n in range(CPC):
            # ---- per-cloud prep ----
            Fsb = clpool.tile([128, HW], F32, tag="Fsb")
            nc.sync.dma_start(Fsb[:], filt_d[n])
            glob_sb = clpool.tile([128, 4], F32, tag="glob")
            nc.sync.dma_start(
                glob_sb[:], glob_d[n].rearrange("(c p) -> p c", p=128))

            # F' = (F * exp(sb2)) transposed -> FT7 [q, c] tiles (bf16)
            FT7 = clpool.tile([128, 896], BF16, tag="FT7")
            for j in range(7):
                q = QT[j]
                ftp = pw.tile([128, 128], F32, tag="w")
                nc.tensor.transpose(
                    ftp[0:q, :], Fsb[:, 128 * j:128 * j + q], ident[:])
                nc.vector.tensor_scalar_mul(
                    FT7[0:q, 128 * j:128 * (j + 1)], ftp[0:q, :],
                    expbf[0:q, j:j + 1])

            # glob matvecs -> per-cloud gate/fold relu biases
            gps = pw.tile([128, 1], F32, tag="w")
            for j in range(4):
                nc.tensor.matmul(
                    gps[:], gw1g[:, 128 * j:128 * (j + 1)],
                    glob_sb[:, j:j + 1], start=(j == 0), stop=(j == 3))
            gbias = clpool.tile([128, 1], F32, tag="gbias")
            nc.vector.tensor_tensor(gbias[:], gps[:], gb1[:], ALU.add)
            fps = pw.tile([128, 1], F32, tag="w")
            for j in range(4):
                nc.tensor.matmul(
                    fps[:], fw1g[:, 128 * j:128 * (j + 1)],
                    glob_sb[:, j:j + 1], start=(j == 0), stop=(j == 3))
            fbias = clpool.tile([128, 1], F32, tag="fbias")
            nc.vector.tensor_tensor(fbias[:], fps[:], fb1[:], ALU.add)

            opre = clpool.tile([3, P], F32, tag="opre")
            msum = clpool.tile([3, 2 * NPAIR], F32, tag="msum")

            # point-major staging for pts+transform; transposed on the PE
            Xpm = clpool.tile([128, 480], F32, tag="Xpm")
            Xpm3 = Xpm[:, :].rearrange("p (b k) -> p b k", k=15)
            nc.gpsimd.dma_start(
                Xpm3[:, :, 0:3], pts_d[n].rearrange("(b p) k -> p b k", p=128))
            nc.gpsimd.dma_start(
                Xpm3[:, :, 3:15], xf_d[n].rearrange("(b p) k -> p b k", p=128))

            # ---- chunk pairs ----
            for cp in range(NPAIR):
                sh2 = spool.tile([128, 2 * CH], BF16, tag="sh2")
                ghs = []
                X17s = []
                pts32s = []
                for h in range(2):
                    c = 2 * cp + h
                    sl = slice(CH * c, CH * (c + 1))
                    X17 = spool.tile([17, CH], BF16, tag=f"X17_{h}")
                    XT = pw.tile([15, CH], F32, tag="w")
                    for b in range(4):
                        nc.tensor.transpose(
                            XT[:, 128 * b:128 * (b + 1)],
                            Xpm[:, 15 * (4 * c + b):15 * (4 * c + b) + 15],
                            ident[:])
                    nc.vector.tensor_copy(X17[0:15, :], XT[:])
                    pts32 = spool.tile([3, CH], F32, tag=f"pts32_{h}")
                    nc.scalar.copy(pts32[:], XT[0:3, :])
                    nc.sync.dma_start(X17[15:17, :], coordsT_d[:, sl])

                    sh_ps = pw.tile([128, CH], F32, tag="w")
                    nc.tensor.matmul(sh_ps[:], w1aug[:, 0:128], X17[:],
                                     start=True, stop=True)
                    gh_ps = pw.tile([128, CH], F32, tag="w")
                    nc.tensor.matmul(gh_ps[:], w1aug[:, 128:256], X17[:],
                                     start=True, stop=True)
                    nc.vector.tensor_scalar(
                        sh2[:, CH * h:CH * (h + 1)], sh_ps[:], sb1[:], 0.0,
                        ALU.add, ALU.max)
                    gh = spool.tile([128, CH], BF16, tag=f"gh_{h}")
                    nc.vector.tensor_scalar(gh[:], gh_ps[:], gbias[:],
                                            0.0, ALU.add, ALU.max)
                    ghs.append(gh)
                    X17s.append(X17)
                    pts32s.append(pts32)

                # z + exp, pair-wide (N=1024 bf16 moving operand)
                E = epool.tile([128, 7 * 2 * CH], BF16, tag="E")
                for j in range(7):
                    q = QT[j]
                    zt = pz.tile([128, 2 * CH], F32, tag="z")
                    for h in range(2):
                        nc.tensor.matmul(zt[0:q, CH * h:CH * (h + 1)],
                                         sw2[:, 128 * j:128 * j + q],
                                         sh2[:, CH * h:CH * (h + 1)],
                                         start=True, stop=True)
                    nc.scalar.activation(
                        E[0:q, 1024 * j:1024 * (j + 1)], zt[0:q, :], AF.Exp)

                for h in range(2):
                    c = 2 * cp + h
                    sl = slice(CH * c, CH * (c + 1))
                    X17, gh, pts32 = X17s[h], ghs[h], pts32s[h]

                    spat = pw.tile([128, CH], F32, tag="w")
                    sume = psm.tile([1, CH], F32, tag="sm")
                    for j in range(7):
                        q = QT[j]
                        esl = E[0:q, 1024 * j + CH * h:1024 * j + CH * (h + 1)]
                        nc.tensor.matmul(
                            spat[:], FT7[0:q, 128 * j:128 * (j + 1)], esl,
                            start=(j == 0), stop=(j == 6))
                        nc.tensor.matmul(
                            sume[:], expb[0:q, j:j + 1], esl,
                            start=(j == 0), stop=(j == 6))

                    ga = pw.tile([128, CH], F32, tag="w")
                    nc.tensor.matmul(ga[:], gw2[:], gh[:],
                                     start=True, stop=True)
                    gt = spool.tile([128, CH], F32, tag="gt")
                    nc.scalar.activation(gt[:], ga[:], AF.Tanh,
                                         bias=gb2h[:], scale=0.5)

                    rinv = spool.tile([1, CH], BF16, tag="rinv")
                    with nc.allow_low_precision(reason="bf16 softmax scale"):
                        nc.vector.reciprocal(rinv[:], sume[:])
                    rbc = psm.tile([128, CH], F32, tag="sm")
                    nc.tensor.matmul(rbc[:], bc05[:], rinv[:],
                                     start=True, stop=True)

                    g1 = spool.tile([128, CH], F32, tag="g1")
                    nc.vector.scalar_tensor_tensor(
                        g1[:], gt[:], 1.0, spat[:], ALU.add, ALU.mult)
                    feats = spool.tile([128, CH], BF16, tag="feats")
                    nc.vector.tensor_tensor(feats[:], g1[:], rbc[:], ALU.mult)

                    f1ps = pw.tile([128, CH], F32, tag="w")
                    nc.tensor.matmul(f1ps[:], w1aug[:, 256:384], X17[:],
                                     start=True, stop=False)
                    nc.tensor.matmul(f1ps[:], fw1s[:], feats[:],
                                     start=False, stop=True)
                    f1 = spool.tile([128, CH], BF16, tag="f1s")
                    nc.vector.tensor_scalar(f1[:], f1ps[:], fbias[:], 0.0,
                                            ALU.add, ALU.max)
                    f2ps = pw.tile([128, CH], F32, tag="w")
                    nc.tensor.matmul(f2ps[:], fw2[:], f1[:],
                                     start=True, stop=True)
                    f2 = spool.tile([128, CH], BF16, tag="f2s")
                    nc.vector.tensor_scalar(f2[:], f2ps[:], fb2[:], 0.0,
                                            ALU.add, ALU.max)
                    f3ps = psm.tile([3, CH], F32, tag="sm")
                    nc.tensor.matmul(f3ps[:], fw3[:], f2[:],
                                     start=True, stop=True)
                    # opre = f3 + fb3 + points (fp32 path); accum -> mean
                    nc.vector.scalar_tensor_tensor(
                        opre[:, sl], f3ps[:], fb3[:], pts32[:],
                        ALU.add, ALU.add, accum_out=msum[:, c:c + 1])

            # ---- per-cloud normalize ----
            msr = spool.tile([3, 1], F32, tag="msr")
            nc.vector.reduce_sum(msr[:], msum[:], axis=mybir.AxisListType.X)
            negmean = spool.tile([3, 1], F32, tag="negmean")
            nc.vector.tensor_scalar_mul(negmean[:], msr[:], -1.0 / P)
            sqc = cl1pool.tile([3, P], F32, tag="sqc")
            nc.scalar.activation(sqc[:], opre[:], AF.Square,
                                 bias=negmean[:], scale=1.0)
            n2 = psm.tile([128, 32], F32, tag="sm")
            for c in range(2 * NPAIR):
                for b in range(4):
                    nc.tensor.matmul(
                        n2[:, 4 * c + b:4 * c + b + 1],
                        sqc[:, 512 * c + 128 * b:512 * c + 128 * (b + 1)],
                        ones3[:], start=True, stop=True)
            nm128 = spool.tile([128, 1], F32, tag="nm128")
            nc.vector.reduce_max(nm128[:], n2[:], axis=mybir.AxisListType.X)
            nmT = psm.tile([1, 128], F32, tag="sm")
            nc.tensor.transpose(nmT[:], nm128[:], ident[:])
            nmax = spool.tile([1, 1], F32, tag="nmax")
            nc.vector.reduce_max(nmax[:], nmT[:], axis=mybir.AxisListType.X)

            # Newton rsqrt: y ~= 1/sqrt(nmax)
            ysh = spool.tile([1, 1], U32, tag="ysh")
            nc.vector.tensor_scalar(ysh[:], nmax[:].bitcast(U32), 1, None,
                                    ALU.logical_shift_right)
            y = spool.tile([1, 1], F32, tag="y")
            nc.vector.tensor_tensor(y[:].bitcast(U32), rsqC[:], ysh[:],
                                    ALU.subtract)
            t = spool.tile([1, 1], F32, tag="t")
            for _ in range(4):
                nc.vector.tensor_tensor(t[:], y[:], y[:], ALU.mult)
                nc.vector.tensor_tensor(t[:], t[:], nmax[:], ALU.mult)
                nc.vector.tensor_scalar(t[:], t[:], -0.5, 1.5,
                                        ALU.mult, ALU.add)
                nc.vector.tensor_tensor(y[:], y[:], t[:], ALU.mult)

            rcolps = psm.tile([3, 1], F32, tag="sm")
            nc.tensor.matmul(rcolps[:], ones13[:], y[:],
                             start=True, stop=True)
            rcol = spool.tile([3, 1], F32, tag="rcol")
            nc.vector.tensor_copy(rcol[:], rcolps[:])
            negmr = spool.tile([3, 1], F32, tag="negmr")
            nc.vector.tensor_tensor(negmr[:], negmean[:], rcol[:], ALU.mult)
            fin = cl1pool.tile([3, P], F32, tag="fin")
            nc.scalar.activation(fin[:], opre[:], AF.Identity,
                                 bias=negmr[:], scale=rcol[:])
            nc.sync.dma_start(out_d[n].rearrange("p c -> c p"), fin[:])

    nc.compile()
    return nc


_prog = None


def _get_prog():
    global _prog
    if _prog is None:
        _prog = _build_program()
    return _prog


def _spatial_grid(k):
    xs = np.linspace(-1.0, 1.0, k, dtype=np.float32)
    gx, gy = np.meshgrid(xs, xs, indexing="ij")
    return np.stack([gx.ravel(), gy.ravel()], axis=-1)


def _host_prep(inputs):
    f32 = np.float32
    sw1 = np.asarray(inputs["sw1"], f32)
    sb1 = np.asarray(inputs["sb1"], f32)
    sw2 = np.asarray(inputs["sw2"], f32)
    sb2 = np.asarray(inputs["sb2"], f32)
    gw1 = np.asarray(inputs["gw1"], f32)
    gb1 = np.asarray(inputs["gb1"], f32)
    gw2 = np.asarray(inputs["gw2"], f32)
    gb2 = np.asarray(inputs["gb2"], f32)
    fw1 = np.asarray(inputs["fw1"], f32)
    fb1 = np.asarray(inputs["fb1"], f32)
    fw2 = np.asarray(inputs["fw2"], f32)
    fb2 = np.asarray(inputs["fb2"], f32)
    fw3 = np.asarray(inputs["fw3"], f32)
    fb3 = np.asarray(inputs["fb3"], f32)

    w1aug = np.zeros((17, 384), f32)
    w1aug[0:15, 0:128] = sw1
    w1aug[0:15, 128:256] = gw1[0:15]
    w1aug[0:15, 256:384] = fw1[0:15]
    w1aug[15:17, 256:384] = fw1[527:529]

    expb = np.zeros((128, 7), f32)
    eb = np.exp(sb2).astype(f32)
    for j in range(7):
        q = QT[j]
        expb[0:q, j] = eb[128 * j:128 * j + q]

    gw1g = np.concatenate(
        [gw1[15 + 128 * j:15 + 128 * (j + 1)] for j in range(4)], axis=1)
    fw1g = np.concatenate(
        [fw1[15 + 128 * j:15 + 128 * (j + 1)] for j in range(4)], axis=1)

    coordsT = np.ascontiguousarray(_spatial_grid(K_GRID).T)

    consts = {
        "w1aug": w1aug.astype(BF),
        "sw2": np.ascontiguousarray(sw2).astype(BF),
        "expb": expb.astype(BF),
        "expbf": expb,
        "gw1g": np.ascontiguousarray(gw1g),
        "fw1g": np.ascontiguousarray(fw1g),
        "fw1s": np.ascontiguousarray(fw1[529:657]).astype(BF),
        "gw2": np.ascontiguousarray(gw2).astype(BF),
        "gb2h": (0.5 * gb2).reshape(128, 1),
        "fw2": np.ascontiguousarray(fw2).astype(BF),
        "fb2": fb2.reshape(128, 1),
        "fw3": np.ascontiguousarray(fw3).astype(BF),
        "fb3": fb3.reshape(3, 1),
        "sb1": sb1.reshape(128, 1),
        "gb1": gb1.reshape(128, 1),
        "fb1": fb1.reshape(128, 1),
        "coordsT": coordsT.astype(BF),
        "ident": np.eye(128, dtype=f32),
        "bc05": np.full((1, 128), 0.5, f32).astype(BF),
        "ones13": np.ones((1, 3), f32),
        "ones3": np.ones((3, 1), f32),
    }
    return {k: (np.ascontiguousarray(v) if v.dtype == BF
                else np.ascontiguousarray(v, f32))
            for k, v in consts.items()}


def _in_maps(inputs):
    f32 = np.float32
    pts = np.asarray(inputs["points"], f32)
    xf = np.asarray(inputs["transform"], f32)
    filt = np.asarray(inputs["enc_filters"], f32).reshape(N, C, HW)
    glob = np.asarray(inputs["enc_glob"], f32)
    consts = _host_prep(inputs)
    maps = []
    for i in range(NCORES):
        s = slice(CPC * i, CPC * (i + 1))
        m = {
            "pts": np.ascontiguousarray(pts[s]),
            "xf": np.ascontiguousarray(xf[s]),
            "filt": np.ascontiguousarray(filt[s]),
            "glob": np.ascontiguousarray(glob[s]),
        }
        m.update(consts)
        maps.append(m)
    return maps


def run(inputs, trace=False):
    nc = _get_prog()
    maps = _in_maps(inputs)
    res = bass_utils.run_bass_kernel_spmd(
        nc, maps, core_ids=list(range(NCORES)), trace=trace)
    out = np.concatenate([res.results[i]["out"] for i in range(NCORES)],
                         axis=0)
    return out.astype(np.float32), res


def kernel(**inputs):
    out, _ = run(inputs, trace=False)
    return out


# revision 26
# speedup vs baseline: 1.1779x; 1.0689x over previous
"""Trainium2 Bass kernel for nn_AttentionFold (self-contained).

Data-parallel over batch N=16: core i processes clouds {2i, 2i+1}.
Feature-major layout on-chip: activations stored [feature, point].

Key restructurings vs the reference:
  - glob (512-dim) contribution to gate/fold hidden layers is a per-cloud
    constant -> one matvec per cloud, applied as relu bias.
  - softmax bias sb2 folded into the filters: E = exp(z), F' = F * exp(sb2),
    sumexp = exp(sb2) . E  (softmax invariant under this refactoring).
  - sigmoid(x) = 0.5 + 0.5*tanh(x/2) so one ACT table set serves the whole
    kernel (exp_and_others: exp/tanh/relu/identity/square).
  - coords grid is input-independent -> host constant, pre-transposed.
  - normalization scale 1/sqrt(max||c||^2) via DVE Newton rsqrt.
  - matmuls in bf16 (fp32 PSUM accumulate) so weight loads use FWL and hide
    behind the matmul stream; the points passthrough to the output is fp32.
  - all weight constants packed into two SBUF blobs (2 DMAs at startup).
  - output produced point-major on the PE (normalize affine folded into the
    transposing matmuls) so the store DMA uses 12-byte-run descriptors.
"""

from contextlib import ExitStack

import numpy as np
import ml_dtypes

import concourse.bass as bass
import concourse.tile as tile
from concourse import bacc, mybir
from concourse import bass_utils

F32 = mybir.dt.float32
BF16 = mybir.dt.bfloat16
U32 = mybir.dt.uint32
AF = mybir.ActivationFunctionType
ALU = mybir.AluOpType
BF = ml_dtypes.bfloat16

N, P, C, HW, G = 16, 4096, 128, 784, 512
NCORES = 8
CPC = N // NCORES          # clouds per core = 2
CH = 512                   # points per chunk
NPAIR = P // (2 * CH)      # chunk pairs per cloud = 4
QT = [128, 128, 128, 128, 128, 128, 16]   # q tiles of HW=784
K_GRID = 64

# const blob layouts: name -> (rows, col0, cols)
_BF_SPECS = [("w1aug", 17, 384), ("sw2", 128, HW), ("expb", 128, 7),
             ("fw1s", 128, 128), ("gw2", 128, 128), ("fw2", 128, 128),
             ("fw3", 128, 3), ("bc05", 1, 128), ("coordsT", 2, P)]
_F32_SPECS = [("expbf", 128, 7), ("gw1g", 128, 512), ("fw1g", 128, 512),
              ("ident", 128, 128), ("gb2h", 128, 1), ("fb2", 128, 1),
              ("fb3", 3, 1), ("sb1", 128, 1), ("gb1", 128, 1),
              ("fb1", 128, 1), ("ones3", 3, 1), ("ones1", 1, 128),
              ("ones13", 1, 3), ("i3t", 3, 96), ("ri3", 3, 3)]


def _layout(specs):
    out, c = {}, 0
    for name, rows, cols in specs:
        out[name] = (rows, c, cols)
        c += cols
    return out, c


BF_L, BF_COLS = _layout(_BF_SPECS)
F32_L, F32_COLS = _layout(_F32_SPECS)


def _build_program():
    nc = bacc.Bacc("TRN2", target_bir_lowering=False, debug=False,
                   num_devices=NCORES)

    def din(name, shape, dt=F32):
        return nc.dram_tensor(name, shape, dt, kind="ExternalInput").ap()

    pts_d = din("pts", (CPC, P, 3))
    xf_d = din("xf", (CPC, P, 12))
    filt_d = din("filt", (CPC, C, HW))
    glob_d = din("glob", (CPC, G))
    bfb_d = din("bfb", (128, BF_COLS), BF16)
    f32b_d = din("f32b", (128, F32_COLS))
    out_d = nc.dram_tensor("out", (CPC, P, 3), F32, kind="ExternalOutput").ap()

    with tile.TileContext(nc) as tc, ExitStack() as ctx:
        cpool = ctx.enter_context(tc.tile_pool(name="consts", bufs=1))
        clpool = ctx.enter_context(tc.tile_pool(name="cloud", bufs=2))
        cl1pool = ctx.enter_context(tc.tile_pool(name="cloud1", bufs=1))
        spool = ctx.enter_context(tc.tile_pool(name="acts", bufs=2))
        epool = ctx.enter_context(tc.tile_pool(name="e", bufs=2))
        # PSUM: 8 banks = z pair-wide (2 x 1) + w (1 x 4) + sm (1 x 2)
        pz = ctx.enter_context(tc.tile_pool(name="pz", bufs=1, space="PSUM"))
        pw = ctx.enter_context(tc.tile_pool(name="pw", bufs=4, space="PSUM"))
        psm = ctx.enter_context(tc.tile_pool(name="psm", bufs=2, space="PSUM"))

        bfb = cpool.tile([128, BF_COLS], BF16, tag="bfb")
        nc.sync.dma_start(bfb[:], bfb_d[:])
        f32b = cpool.tile([128, F32_COLS], F32, tag="f32b")
        nc.sync.dma_start(f32b[:], f32b_d[:])

        def bfc(name):
            r, c0, cc = BF_L[name]
            return bfb[0:r, c0:c0 + cc]

        def f32c(name):
            r, c0, cc = F32_L[name]
            return f32b[0:r, c0:c0 + cc]

        w1aug, sw2, expb = bfc("w1aug"), bfc("sw2"), bfc("expb")
        fw1s, gw2, fw2, fw3 = bfc("fw1s"), bfc("gw2"), bfc("fw2"), bfc("fw3")
        bc05, coordsT = bfc("bc05"), bfc("coordsT")
        expbf, gw1g, fw1g, ident = (f32c("expbf"), f32c("gw1g"),
                                    f32c("fw1g"), f32c("ident"))
        gb2h, fb2, fb3, sb1, gb1, fb1 = (f32c("gb2h"), f32c("fb2"),
                                         f32c("fb3"), f32c("sb1"),
                                         f32c("gb1"), f32c("fb1"))
        ones3, ones1, i3t, ri3 = (f32c("ones3"), f32c("ones1"),
                                  f32c("i3t"), f32c("ri3"))
        ones13 = f32c("ones13")
        rsqC = cpool.tile([1, 1], U32, tag="rsqC")
        nc.vector.memset(rsqC[:], 0x5F3759DF)

        for n in range(CPC):
            # ---- per-cloud prep ----
            Fsb = clpool.tile([128, HW], F32, tag="Fsb")
            nc.sync.dma_start(Fsb[:], filt_d[n])
            glob_sb = clpool.tile([128, 4], F32, tag="glob")
            nc.sync.dma_start(
                glob_sb[:], glob_d[n].rearrange("(c p) -> p c", p=128))

            # F' = (F * exp(sb2)) transposed -> FT7 [q, c] tiles (bf16)
            FT7 = clpool.tile([128, 896], BF16, tag="FT7")
            for j in range(7):
                q = QT[j]
                ftp = pw.tile([128, 128], F32, tag="w")
                nc.tensor.transpose(
                    ftp[0:q, :], Fsb[:, 128 * j:128 * j + q], ident)
                nc.vector.tensor_scalar_mul(
                    FT7[0:q, 128 * j:128 * (j + 1)], ftp[0:q, :],
                    expbf[0:q, j:j + 1])

            # glob matvecs -> per-cloud gate/fold relu biases
            gps = pw.tile([128, 1], F32, tag="w")
            for j in range(4):
                nc.tensor.matmul(
                    gps[:], gw1g[:, 128 * j:128 * (j + 1)],
                    glob_sb[:, j:j + 1], start=(j == 0), stop=(j == 3))
            gbias = clpool.tile([128, 1], F32, tag="gbias")
            nc.vector.tensor_tensor(gbias[:], gps[:], gb1, ALU.add)
            fps = pw.tile([128, 1], F32, tag="w")
            for j in range(4):
                nc.tensor.matmul(
                    fps[:], fw1g[:, 128 * j:128 * (j + 1)],
                    glob_sb[:, j:j + 1], start=(j == 0), stop=(j == 3))
            fbias = clpool.tile([128, 1], F32, tag="fbias")
            nc.vector.tensor_tensor(fbias[:], fps[:], fb1, ALU.add)

            opre = clpool.tile([3, P], F32, tag="opre")
            msum = clpool.tile([3, 2 * NPAIR], F32, tag="msum")

            # point-major staging for pts+transform -> feature-major via PE
            Xpm = clpool.tile([128, 480], F32, tag="Xpm")
            Xpm3 = Xpm[:, :].rearrange("p (b k) -> p b k", k=15)
            nc.gpsimd.dma_start(
                Xpm3[:, :, 0:3], pts_d[n].rearrange("(b p) k -> p b k", p=128))
            nc.gpsimd.dma_start(
                Xpm3[:, :, 3:15], xf_d[n].rearrange("(b p) k -> p b k", p=128))

            X17c = clpool.tile([17, P], BF16, tag="X17c")
            pts32c = clpool.tile([3, P], F32, tag="pts32c")
            for c in range(2 * NPAIR):
                XT = pw.tile([15, CH], F32, tag="w")
                for b in range(4):
                    nc.tensor.transpose(
                        XT[:, 128 * b:128 * (b + 1)],
                        Xpm[:, 15 * (4 * c + b):15 * (4 * c + b) + 15],
                        ident)
                sl = slice(CH * c, CH * (c + 1))
                nc.vector.tensor_copy(X17c[0:15, sl], XT[:])
                nc.scalar.copy(pts32c[:, sl], XT[0:3, :])
            nc.sync.dma_start(X17c[15:17, :], coordsT)

            # ---- chunk pairs ----
            for cp in range(NPAIR):
                sh2 = spool.tile([128, 2 * CH], BF16, tag="sh2")
                ghs = []
                for h in range(2):
                    c = 2 * cp + h
                    sl = slice(CH * c, CH * (c + 1))
                    sh_ps = pw.tile([128, CH], F32, tag="w")
                    nc.tensor.matmul(sh_ps[:], w1aug[:, 0:128], X17c[:, sl],
                                     start=True, stop=True)
                    gh_ps = pw.tile([128, CH], F32, tag="w")
                    nc.tensor.matmul(gh_ps[:], w1aug[:, 128:256], X17c[:, sl],
                                     start=True, stop=True)
                    nc.vector.tensor_scalar(
                        sh2[:, CH * h:CH * (h + 1)], sh_ps[:], sb1, 0.0,
                        ALU.add, ALU.max)
                    gh = spool.tile([128, CH], BF16, tag=f"gh_{h}")
                    nc.vector.tensor_scalar(gh[:], gh_ps[:], gbias[:],
                                            0.0, ALU.add, ALU.max)
                    ghs.append(gh)

                # z + exp, pair-wide
                E = epool.tile([128, 7 * 2 * CH], BF16, tag="E")
                for j in range(7):
                    q = QT[j]
                    zt = pz.tile([128, 2 * CH], F32, tag="z")
                    for h in range(2):
                        nc.tensor.matmul(zt[0:q, CH * h:CH * (h + 1)],
                                         sw2[:, 128 * j:128 * j + q],
                                         sh2[:, CH * h:CH * (h + 1)],
                                         start=True, stop=True)
                    nc.scalar.activation(
                        E[0:q, 1024 * j:1024 * (j + 1)], zt[0:q, :], AF.Exp)

                for h in range(2):
                    c = 2 * cp + h
                    sl = slice(CH * c, CH * (c + 1))
                    gh = ghs[h]

                    spat = pw.tile([128, CH], F32, tag="w")
                    sume = psm.tile([1, CH], F32, tag="sm")
                    for j in range(7):
                        q = QT[j]
                        esl = E[0:q, 1024 * j + CH * h:1024 * j + CH * (h + 1)]
                        nc.tensor.matmul(
                            spat[:], FT7[0:q, 128 * j:128 * (j + 1)], esl,
                            start=(j == 0), stop=(j == 6))
                        nc.tensor.matmul(
                            sume[:], expb[0:q, j:j + 1], esl,
                            start=(j == 0), stop=(j == 6))

                    ga = pw.tile([128, CH], F32, tag="w")
                    nc.tensor.matmul(ga[:], gw2, gh[:], start=True, stop=True)
                    gt = spool.tile([128, CH], F32, tag="gt")
                    nc.scalar.activation(gt[:], ga[:], AF.Tanh,
                                         bias=gb2h, scale=0.5)

                    rinv = spool.tile([1, CH], BF16, tag="rinv")
                    with nc.allow_low_precision(reason="bf16 softmax scale"):
                        nc.vector.reciprocal(rinv[:], sume[:])
                    rbc = psm.tile([128, CH], F32, tag="sm")
                    nc.tensor.matmul(rbc[:], bc05, rinv[:],
                                     start=True, stop=True)

                    g1 = spool.tile([128, CH], F32, tag="g1")
                    nc.vector.scalar_tensor_tensor(
                        g1[:], gt[:], 1.0, spat[:], ALU.add, ALU.mult)
                    feats = spool.tile([128, CH], BF16, tag="feats")
                    nc.vector.tensor_tensor(feats[:], g1[:], rbc[:], ALU.mult)

                    f1ps = pw.tile([128, CH], F32, tag="w")
                    nc.tensor.matmul(f1ps[:], w1aug[:, 256:384], X17c[:, sl],
                                     start=True, stop=False)
                    nc.tensor.matmul(f1ps[:], fw1s, feats[:],
                                     start=False, stop=True)
                    f1 = spool.tile([128, CH], BF16, tag="f1s")
                    nc.vector.tensor_scalar(f1[:], f1ps[:], fbias[:], 0.0,
                                            ALU.add, ALU.max)
                    f2ps = pw.tile([128, CH], F32, tag="w")
                    nc.tensor.matmul(f2ps[:], fw2, f1[:],
                                     start=True, stop=True)
                    f2 = spool.tile([128, CH], BF16, tag="f2s")
                    nc.vector.tensor_scalar(f2[:], f2ps[:], fb2, 0.0,
                                            ALU.add, ALU.max)
                    f3ps = psm.tile([3, CH], F32, tag="sm")
                    nc.tensor.matmul(f3ps[:], fw3, f2[:],
                                     start=True, stop=True)
                    # opre = f3 + fb3 + points (fp32 path); accum -> mean
                    nc.vector.scalar_tensor_tensor(
                        opre[:, sl], f3ps[:], fb3, pts32c[:, sl],
                        ALU.add, ALU.add, accum_out=msum[:, c:c + 1])

            # ---- per-cloud normalize ----
            msr = spool.tile([3, 1], F32, tag="msr")
            nc.vector.reduce_sum(msr[:], msum[:], axis=mybir.AxisListType.X)
            negmean = spool.tile([3, 1], F32, tag="negmean")
            nc.vector.tensor_scalar_mul(negmean[:], msr[:], -1.0 / P)
            sqc = cl1pool.tile([3, P], F32, tag="sqc")
            nc.scalar.activation(sqc[:], opre[:], AF.Square,
                                 bias=negmean[:], scale=1.0)
            n2 = psm.tile([128, 32], F32, tag="sm")
            for c in range(2 * NPAIR):
                for b in range(4):
                    nc.tensor.matmul(
                        n2[:, 4 * c + b:4 * c + b + 1],
                        sqc[:, 512 * c + 128 * b:512 * c + 128 * (b + 1)],
                        ones3, start=True, stop=True)
            nm128 = spool.tile([128, 1], F32, tag="nm128")
            nc.vector.reduce_max(nm128[:], n2[:], axis=mybir.AxisListType.X)
            nmT = psm.tile([1, 128], F32, tag="sm")
            nc.tensor.transpose(nmT[:], nm128[:], ident)
            nmax = spool.tile([1, 1], F32, tag="nmax")
            nc.vector.reduce_max(nmax[:], nmT[:], axis=mybir.AxisListType.X)

            # Newton rsqrt: y ~= 1/sqrt(nmax)
            ysh = spool.tile([1, 1], U32, tag="ysh")
            nc.vector.tensor_scalar(ysh[:], nmax[:].bitcast(U32), 1, None,
                                    ALU.logical_shift_right)
            y = spool.tile([1, 1], F32, tag="y")
            nc.vector.tensor_tensor(y[:].bitcast(U32), rsqC[:], ysh[:],
                                    ALU.subtract)
            t = spool.tile([1, 1], F32, tag="t")
            for _ in range(4):
                nc.vector.tensor_tensor(t[:], y[:], y[:], ALU.mult)
                nc.vector.tensor_tensor(t[:], t[:], nmax[:], ALU.mult)
                nc.vector.tensor_scalar(t[:], t[:], -0.5, 1.5,
                                        ALU.mult, ALU.add)
                nc.vector.tensor_tensor(y[:], y[:], t[:], ALU.mult)

            # rcol = y broadcast to 3 partitions
            rcolps = psm.tile([3, 1], F32, tag="sm")
            nc.tensor.matmul(rcolps[:], ones13, y[:], start=True, stop=True)
            rcol = spool.tile([3, 1], F32, tag="rcol")
            nc.vector.tensor_copy(rcol[:], rcolps[:])
            # rI3 = y * I3 ; negmr96 = tiled row of (negmean * y)
            rI3 = spool.tile([3, 3], F32, tag="rI3")
            nc.vector.tensor_scalar_mul(rI3[:], ri3, rcol[:])
            negmr = spool.tile([3, 1], F32, tag="negmr")
            nc.vector.tensor_scalar_mul(negmr[:], negmean[:], rcol[:])
            nm96ps = psm.tile([1, 96], F32, tag="sm")
            nc.tensor.matmul(nm96ps[:], negmr[:], i3t, start=True, stop=True)
            negmr96 = spool.tile([1, 96], F32, tag="negmr96")
            nc.vector.tensor_copy(negmr96[:], nm96ps[:])

            # point-major output: fin_pm[p, 3b:3b+3] = r*opre.T + bias
            finpm_ps = psm.tile([128, 96], F32, tag="sm")
            fin = cl1pool.tile([128, 96], F32, tag="fin")
            for g in range(8):
                gsl = slice(12 * g, 12 * (g + 1))
                nc.tensor.matmul(finpm_ps[:, gsl], ones1,
                                 negmr96[:, gsl], start=True, stop=False,
                                 skip_group_check=True)
                for bi in range(4):
                    b = 4 * g + bi
                    nc.tensor.matmul(
                        finpm_ps[:, 3 * b:3 * (b + 1)],
                        opre[:, 128 * b:128 * (b + 1)], rI3[:],
                        start=False, stop=(bi == 3), skip_group_check=True)
                nc.vector.tensor_copy(fin[:, gsl], finpm_ps[:, gsl])
            nc.sync.dma_start(
                out_d[n].rearrange("(b p) c -> p b c", p=128),
                fin[:, :].rearrange("p (b c) -> p b c", c=3))

    nc.compile()
    return nc


_prog = None


def _get_prog():
    global _prog
    if _prog is None:
        _prog = _build_program()
    return _prog


def _spatial_grid(k):
    xs = np.linspace(-1.0, 1.0, k, dtype=np.float32)
    gx, gy = np.meshgrid(xs, xs, indexing="ij")
    return np.stack([gx.ravel(), gy.ravel()], axis=-1)


def _host_prep(inputs):
    f32 = np.float32
    sw1 = np.asarray(inputs["sw1"], f32)
    sb1 = np.asarray(inputs["sb1"], f32)
    sw2 = np.asarray(inputs["sw2"], f32)
    sb2 = np.asarray(inputs["sb2"], f32)
    gw1 = np.asarray(inputs["gw1"], f32)
    gb1 = np.asarray(inputs["gb1"], f32)
    gw2 = np.asarray(inputs["gw2"], f32)
    gb2 = np.asarray(inputs["gb2"], f32)
    fw1 = np.asarray(inputs["fw1"], f32)
    fb1 = np.asarray(inputs["fb1"], f32)
    fw2 = np.asarray(inputs["fw2"], f32)
    fb2 = np.asarray(inputs["fb2"], f32)
    fw3 = np.asarray(inputs["fw3"], f32)
    fb3 = np.asarray(inputs["fb3"], f32)

    w1aug = np.zeros((17, 384), f32)
    w1aug[0:15, 0:128] = sw1
    w1aug[0:15, 128:256] = gw1[0:15]
    w1aug[0:15, 256:384] = fw1[0:15]
    w1aug[15:17, 256:384] = fw1[527:529]

    expb = np.zeros((128, 7), f32)
    eb = np.exp(sb2).astype(f32)
    for j in range(7):
        q = QT[j]
        expb[0:q, j] = eb[128 * j:128 * j + q]

    gw1g = np.concatenate(
        [gw1[15 + 128 * j:15 + 128 * (j + 1)] for j in range(4)], axis=1)
    fw1g = np.concatenate(
        [fw1[15 + 128 * j:15 + 128 * (j + 1)] for j in range(4)], axis=1)

    vals = {
        "w1aug": w1aug,
        "sw2": sw2,
        "expb": expb,
        "expbf": expb,
        "gw1g": gw1g,
        "fw1g": fw1g,
        "fw1s": fw1[529:657],
        "gw2": gw2,
        "gb2h": (0.5 * gb2).reshape(128, 1),
        "fw2": fw2,
        "fb2": fb2.reshape(128, 1),
        "fw3": fw3,
        "fb3": fb3.reshape(3, 1),
        "sb1": sb1.reshape(128, 1),
        "gb1": gb1.reshape(128, 1),
        "fb1": fb1.reshape(128, 1),
        "coordsT": _spatial_grid(K_GRID).T,
        "ident": np.eye(128, dtype=f32),
        "bc05": np.full((1, 128), 0.5, f32),
        "ones3": np.ones((3, 1), f32),
        "ones1": np.ones((1, 128), f32),
        "ones13": np.ones((1, 3), f32),
        "i3t": np.tile(np.eye(3, dtype=f32), (1, 32)),
        "ri3": np.eye(3, dtype=f32),
    }

    def pack(specs, layout, ncols, dtype):
        blob = np.zeros((128, ncols), dtype)
        for name, rows, cols in specs:
            r, c0, cc = layout[name]
            blob[0:r, c0:c0 + cc] = np.asarray(vals[name], f32).astype(dtype)
        return blob

    return {
        "bfb": pack(_BF_SPECS, BF_L, BF_COLS, BF),
        "f32b": pack(_F32_SPECS, F32_L, F32_COLS, f32),
    }


def _in_maps(inputs):
    f32 = np.float32
    pts = np.asarray(inputs["points"], f32)
    xf = np.asarray(inputs["transform"], f32)
    filt = np.asarray(inputs["enc_filters"], f32).reshape(N, C, HW)
    glob = np.asarray(inputs["enc_glob"], f32)
    consts = _host_prep(inputs)
    maps = []
    for i in range(NCORES):
        s = slice(CPC * i, CPC * (i + 1))
        m = {
            "pts": np.ascontiguousarray(pts[s]),
            "xf": np.ascontiguousarray(xf[s]),
            "filt": np.ascontiguousarray(filt[s]),
            "glob": np.ascontiguousarray(glob[s]),
        }
        m.update(consts)
        maps.append(m)
    return maps


def run(inputs, trace=False):
    nc = _get_prog()
    maps = _in_maps(inputs)
    res = bass_utils.run_bass_kernel_spmd(
        nc, maps, core_ids=list(range(NCORES)), trace=trace)
    out = np.concatenate([res.results[i]["out"] for i in range(NCORES)],
                         axis=0)
    return out.astype(np.float32), res


def kernel(**inputs):
    out, _ = run(inputs, trace=False)
    return out


# revision 28
# speedup vs baseline: 1.2952x; 1.0996x over previous
"""Trainium2 Bass kernel for nn_AttentionFold (self-contained).

Data-parallel over batch N=16: core i processes clouds {2i, 2i+1}.
Feature-major layout on-chip: activations stored [feature, point].

Key restructurings vs the reference:
  - glob (512-dim) contribution to gate/fold hidden layers is a per-cloud
    constant -> one matvec per cloud, applied as relu bias.
  - softmax bias sb2 folded into the filters: E = exp(z), F' = F * exp(sb2),
    sumexp = exp(sb2) . E  (softmax invariant under this refactoring).
  - sigmoid(x) = 0.5 + 0.5*tanh(x/2) so one ACT table set serves the whole
    kernel (exp_and_others: exp/tanh/relu/identity/square).
  - coords grid is input-independent -> host constant, pre-transposed.
  - normalization scale 1/sqrt(max||c||^2) via DVE Newton rsqrt.
  - matmuls in bf16 (fp32 PSUM accumulate) so weight loads use FWL and hide
    behind the matmul stream; the points passthrough to the output is fp32.
  - all weight constants packed into two SBUF blobs (2 DMAs at startup).
  - output produced point-major on the PE (normalize affine folded into the
    transposing matmuls) so the store DMA uses 12-byte-run descriptors.
"""

from contextlib import ExitStack

import numpy as np
import ml_dtypes

import concourse.bass as bass
import concourse.tile as tile
from concourse import bacc, mybir
from concourse import bass_utils

F32 = mybir.dt.float32
BF16 = mybir.dt.bfloat16
U32 = mybir.dt.uint32
AF = mybir.ActivationFunctionType
ALU = mybir.AluOpType
BF = ml_dtypes.bfloat16

N, P, C, HW, G = 16, 4096, 128, 784, 512
NCORES = 8
CPC = N // NCORES          # clouds per core = 2
CH = 512                   # points per chunk
NPAIR = P // (2 * CH)      # chunk pairs per cloud = 4
QT = [128, 128, 128, 128, 128, 128, 16]   # q tiles of HW=784
K_GRID = 64

# const blob layouts: name -> (rows, col0, cols)
_BF_SPECS = [("w1aug", 17, 384), ("sw2", 128, HW), ("expb", 128, 7),
             ("fw1s", 128, 128), ("gw2", 128, 128), ("fw2", 128, 128),
             ("fw3", 128, 3), ("bc05", 1, 128), ("coordsT", 2, P)]
_F32_SPECS = [("expbf", 128, 7), ("gw1g", 128, 512), ("fw1g", 128, 512),
              ("ident", 128, 128), ("gb2h", 128, 1), ("fb2", 128, 1),
              ("fb3", 3, 1), ("sb1", 128, 1), ("gb1", 128, 1),
              ("fb1", 128, 1), ("ones3", 3, 1), ("ones1", 1, 128),
              ("ones13", 1, 3), ("i3t", 3, 96), ("ri3", 3, 3)]


def _layout(specs):
    out, c = {}, 0
    for name, rows, cols in specs:
        out[name] = (rows, c, cols)
        c += cols
    return out, c


BF_L, BF_COLS = _layout(_BF_SPECS)
F32_L, F32_COLS = _layout(_F32_SPECS)


def _build_program():
    nc = bacc.Bacc("TRN2", target_bir_lowering=False, debug=False,
                   num_devices=NCORES)

    def din(name, shape, dt=F32):
        return nc.dram_tensor(name, shape, dt, kind="ExternalInput").ap()

    pts_d = din("pts", (CPC, P, 3))
    xf_d = din("xf", (CPC, P, 12))
    filt_d = din("filt", (CPC, C, HW))
    glob_d = din("glob", (CPC, G))
    bfb_d = din("bfb", (128, BF_COLS), BF16)
    f32b_d = din("f32b", (128, F32_COLS))
    out_d = nc.dram_tensor("out", (CPC, P, 3), F32, kind="ExternalOutput").ap()

    with tile.TileContext(nc) as tc, ExitStack() as ctx:
        cpool = ctx.enter_context(tc.tile_pool(name="consts", bufs=1))
        clpool = ctx.enter_context(tc.tile_pool(name="cloud", bufs=2))
        cl1pool = ctx.enter_context(tc.tile_pool(name="cloud1", bufs=1))
        spool = ctx.enter_context(tc.tile_pool(name="acts", bufs=2))
        epool = ctx.enter_context(tc.tile_pool(name="e", bufs=2))
        # PSUM: 8 banks = z pair-wide (2 x 1) + w (1 x 4) + sm (1 x 2)
        pz = ctx.enter_context(tc.tile_pool(name="pz", bufs=1, space="PSUM"))
        pw = ctx.enter_context(tc.tile_pool(name="pw", bufs=4, space="PSUM"))
        psm = ctx.enter_context(tc.tile_pool(name="psm", bufs=2, space="PSUM"))

        bfb = cpool.tile([128, BF_COLS], BF16, tag="bfb")
        nc.sync.dma_start(bfb[:], bfb_d[:])
        f32b = cpool.tile([128, F32_COLS], F32, tag="f32b")
        nc.sync.dma_start(f32b[:], f32b_d[:])

        def bfc(name):
            r, c0, cc = BF_L[name]
            return bfb[0:r, c0:c0 + cc]

        def f32c(name):
            r, c0, cc = F32_L[name]
            return f32b[0:r, c0:c0 + cc]

        w1aug, sw2, expb = bfc("w1aug"), bfc("sw2"), bfc("expb")
        fw1s, gw2, fw2, fw3 = bfc("fw1s"), bfc("gw2"), bfc("fw2"), bfc("fw3")
        bc05, coordsT = bfc("bc05"), bfc("coordsT")
        expbf, gw1g, fw1g, ident = (f32c("expbf"), f32c("gw1g"),
                                    f32c("fw1g"), f32c("ident"))
        gb2h, fb2, fb3, sb1, gb1, fb1 = (f32c("gb2h"), f32c("fb2"),
                                         f32c("fb3"), f32c("sb1"),
                                         f32c("gb1"), f32c("fb1"))
        ones3, ones1, i3t, ri3 = (f32c("ones3"), f32c("ones1"),
                                  f32c("i3t"), f32c("ri3"))
        ones13 = f32c("ones13")
        rsqC = cpool.tile([1, 1], U32, tag="rsqC")
        nc.vector.memset(rsqC[:], 0x5F3759DF)

        for n in range(CPC):
            # ---- per-cloud prep ----
            Fsb = clpool.tile([128, HW], F32, tag="Fsb")
            nc.sync.dma_start(Fsb[:], filt_d[n])
            glob_sb = clpool.tile([128, 4], F32, tag="glob")
            nc.sync.dma_start(
                glob_sb[:], glob_d[n].rearrange("(c p) -> p c", p=128))

            # F' = (F * exp(sb2)) transposed -> FT7 [q, c] tiles (bf16)
            FT7 = clpool.tile([128, 896], BF16, tag="FT7")
            for j in range(7):
                q = QT[j]
                ftp = pw.tile([128, 128], F32, tag="w")
                nc.tensor.transpose(
                    ftp[0:q, :], Fsb[:, 128 * j:128 * j + q], ident)
                nc.vector.tensor_scalar_mul(
                    FT7[0:q, 128 * j:128 * (j + 1)], ftp[0:q, :],
                    expbf[0:q, j:j + 1])

            # glob matvecs -> per-cloud gate/fold relu biases
            gps = pw.tile([128, 1], F32, tag="w")
            for j in range(4):
                nc.tensor.matmul(
                    gps[:], gw1g[:, 128 * j:128 * (j + 1)],
                    glob_sb[:, j:j + 1], start=(j == 0), stop=(j == 3))
            gbias = clpool.tile([128, 1], F32, tag="gbias")
            nc.vector.tensor_tensor(gbias[:], gps[:], gb1, ALU.add)
            fps = pw.tile([128, 1], F32, tag="w")
            for j in range(4):
                nc.tensor.matmul(
                    fps[:], fw1g[:, 128 * j:128 * (j + 1)],
                    glob_sb[:, j:j + 1], start=(j == 0), stop=(j == 3))
            fbias = clpool.tile([128, 1], F32, tag="fbias")
            nc.vector.tensor_tensor(fbias[:], fps[:], fb1, ALU.add)

            opre = clpool.tile([3, P], F32, tag="opre")
            msum = clpool.tile([3, 2 * NPAIR], F32, tag="msum")

            # point-major staging for pts+transform -> feature-major via PE
            Xpm = clpool.tile([128, 480], F32, tag="Xpm")
            Xpm3 = Xpm[:, :].rearrange("p (b k) -> p b k", k=15)
            nc.gpsimd.dma_start(
                Xpm3[:, :, 0:3], pts_d[n].rearrange("(b p) k -> p b k", p=128))
            nc.gpsimd.dma_start(
                Xpm3[:, :, 3:15], xf_d[n].rearrange("(b p) k -> p b k", p=128))

            X17c = clpool.tile([17, P], BF16, tag="X17c")
            pts32c = clpool.tile([3, P], F32, tag="pts32c")
            for c in range(2 * NPAIR):
                XT = pw.tile([15, CH], F32, tag="w")
                for b in range(4):
                    nc.tensor.transpose(
                        XT[:, 128 * b:128 * (b + 1)],
                        Xpm[:, 15 * (4 * c + b):15 * (4 * c + b) + 15],
                        ident)
                sl = slice(CH * c, CH * (c + 1))
                nc.vector.tensor_copy(X17c[0:15, sl], XT[:])
                nc.scalar.copy(pts32c[:, sl], XT[0:3, :])
            nc.sync.dma_start(X17c[15:17, :], coordsT)

            # ---- chunk pairs ----
            for cp in range(NPAIR):
                sh2 = spool.tile([128, 2 * CH], BF16, tag="sh2")
                ghs = []
                for h in range(2):
                    c = 2 * cp + h
                    sl = slice(CH * c, CH * (c + 1))
                    sh_ps = pw.tile([128, CH], F32, tag="w")
                    nc.tensor.matmul(sh_ps[:], w1aug[:, 0:128], X17c[:, sl],
                                     start=True, stop=True)
                    gh_ps = pw.tile([128, CH], F32, tag="w")
                    nc.tensor.matmul(gh_ps[:], w1aug[:, 128:256], X17c[:, sl],
                                     start=True, stop=True)
                    nc.vector.tensor_scalar(
                        sh2[:, CH * h:CH * (h + 1)], sh_ps[:], sb1, 0.0,
                        ALU.add, ALU.max)
                    gh = spool.tile([128, CH], BF16, tag=f"gh_{h}")
                    nc.vector.tensor_scalar(gh[:], gh_ps[:], gbias[:],
                                            0.0, ALU.add, ALU.max)
                    ghs.append(gh)

                # gate second layer early (PE filler during exp rounds)
                gts = []
                for h in range(2):
                    ga = pw.tile([128, CH], F32, tag="w")
                    nc.tensor.matmul(ga[:], gw2, ghs[h][:],
                                     start=True, stop=True)
                    gt = spool.tile([128, CH], F32, tag=f"gt_{h}")
                    nc.scalar.activation(gt[:], ga[:], AF.Tanh,
                                         bias=gb2h, scale=0.5)
                    gts.append(gt)

                # z/exp rounds interleaved with the previous round's attention
                E = epool.tile([128, 7 * 2 * CH], BF16, tag="E")
                spat0 = pw.tile([128, CH], F32, tag="w")
                spat1 = pw.tile([128, CH], F32, tag="w")
                sume0 = psm.tile([1, CH], F32, tag="sm")
                sume1 = psm.tile([1, CH], F32, tag="sm")
                spats = [spat0, spat1]
                sumes = [sume0, sume1]

                def attn_round(j):
                    q = QT[j]
                    for h in range(2):
                        esl = E[0:q, 1024 * j + CH * h:1024 * j + CH * (h + 1)]
                        nc.tensor.matmul(
                            spats[h][:], FT7[0:q, 128 * j:128 * (j + 1)], esl,
                            start=(j == 0), stop=(j == 6))
                        nc.tensor.matmul(
                            sumes[h][:], expb[0:q, j:j + 1], esl,
                            start=(j == 0), stop=(j == 6))

                for j in range(7):
                    q = QT[j]
                    zt = pz.tile([128, 2 * CH], F32, tag="z")
                    for h in range(2):
                        nc.tensor.matmul(zt[0:q, CH * h:CH * (h + 1)],
                                         sw2[:, 128 * j:128 * j + q],
                                         sh2[:, CH * h:CH * (h + 1)],
                                         start=True, stop=True)
                    nc.scalar.activation(
                        E[0:q, 1024 * j:1024 * (j + 1)], zt[0:q, :], AF.Exp)
                    if j > 0:
                        attn_round(j - 1)
                attn_round(6)

                for h in range(2):
                    c = 2 * cp + h
                    sl = slice(CH * c, CH * (c + 1))
                    spat, sume, gt = spats[h], sumes[h], gts[h]

                    rinv = spool.tile([1, CH], BF16, tag="rinv")
                    with nc.allow_low_precision(reason="bf16 softmax scale"):
                        nc.vector.reciprocal(rinv[:], sume[:])
                    rbc = psm.tile([128, CH], F32, tag="sm")
                    nc.tensor.matmul(rbc[:], bc05, rinv[:],
                                     start=True, stop=True)

                    g1 = spool.tile([128, CH], F32, tag="g1")
                    nc.vector.scalar_tensor_tensor(
                        g1[:], gt[:], 1.0, spat[:], ALU.add, ALU.mult)
                    feats = spool.tile([128, CH], BF16, tag="feats")
                    nc.vector.tensor_tensor(feats[:], g1[:], rbc[:], ALU.mult)

                    f1ps = pw.tile([128, CH], F32, tag="w")
                    nc.tensor.matmul(f1ps[:], w1aug[:, 256:384], X17c[:, sl],
                                     start=True, stop=False)
                    nc.tensor.matmul(f1ps[:], fw1s, feats[:],
                                     start=False, stop=True)
                    f1 = spool.tile([128, CH], BF16, tag="f1s")
                    nc.vector.tensor_scalar(f1[:], f1ps[:], fbias[:], 0.0,
                                            ALU.add, ALU.max)
                    f2ps = pw.tile([128, CH], F32, tag="w")
                    nc.tensor.matmul(f2ps[:], fw2, f1[:],
                                     start=True, stop=True)
                    f2 = spool.tile([128, CH], BF16, tag="f2s")
                    nc.vector.tensor_scalar(f2[:], f2ps[:], fb2, 0.0,
                                            ALU.add, ALU.max)
                    f3ps = psm.tile([3, CH], F32, tag="sm")
                    nc.tensor.matmul(f3ps[:], fw3, f2[:],
                                     start=True, stop=True)
                    # opre = f3 + fb3 + points (fp32 path); accum -> mean
                    nc.vector.scalar_tensor_tensor(
                        opre[:, sl], f3ps[:], fb3, pts32c[:, sl],
                        ALU.add, ALU.add, accum_out=msum[:, c:c + 1])

            # ---- per-cloud normalize ----
            msr = spool.tile([3, 1], F32, tag="msr")
            nc.vector.reduce_sum(msr[:], msum[:], axis=mybir.AxisListType.X)
            negmean = spool.tile([3, 1], F32, tag="negmean")
            nc.vector.tensor_scalar_mul(negmean[:], msr[:], -1.0 / P)
            sqc = cl1pool.tile([3, P], F32, tag="sqc")
            nc.scalar.activation(sqc[:], opre[:], AF.Square,
                                 bias=negmean[:], scale=1.0)
            n2 = psm.tile([128, 32], F32, tag="sm")
            for c in range(2 * NPAIR):
                for b in range(4):
                    nc.tensor.matmul(
                        n2[:, 4 * c + b:4 * c + b + 1],
                        sqc[:, 512 * c + 128 * b:512 * c + 128 * (b + 1)],
                        ones3, start=True, stop=True)
            nm128 = spool.tile([128, 1], F32, tag="nm128")
            nc.vector.reduce_max(nm128[:], n2[:], axis=mybir.AxisListType.X)
            nmT = psm.tile([1, 128], F32, tag="sm")
            nc.tensor.transpose(nmT[:], nm128[:], ident)
            nmax = spool.tile([1, 1], F32, tag="nmax")
            nc.vector.reduce_max(nmax[:], nmT[:], axis=mybir.AxisListType.X)

            # Newton rsqrt: y ~= 1/sqrt(nmax)
            ysh = spool.tile([1, 1], U32, tag="ysh")
            nc.vector.tensor_scalar(ysh[:], nmax[:].bitcast(U32), 1, None,
                                    ALU.logical_shift_right)
            y = spool.tile([1, 1], F32, tag="y")
            nc.vector.tensor_tensor(y[:].bitcast(U32), rsqC[:], ysh[:],
                                    ALU.subtract)
            t = spool.tile([1, 1], F32, tag="t")
            for _ in range(4):
                nc.vector.tensor_tensor(t[:], y[:], y[:], ALU.mult)
                nc.vector.tensor_tensor(t[:], t[:], nmax[:], ALU.mult)
                nc.vector.tensor_scalar(t[:], t[:], -0.5, 1.5,
                                        ALU.mult, ALU.add)
                nc.vector.tensor_tensor(y[:], y[:], t[:], ALU.mult)

            # rcol = y broadcast to 3 partitions
            rcolps = psm.tile([3, 1], F32, tag="sm")
            nc.tensor.matmul(rcolps[:], ones13, y[:], start=True, stop=True)
            rcol = spool.tile([3, 1], F32, tag="rcol")
            nc.vector.tensor_copy(rcol[:], rcolps[:])
            # rI3 = y * I3 ; negmr96 = tiled row of (negmean * y)
            rI3 = spool.tile([3, 3], F32, tag="rI3")
            nc.vector.tensor_scalar_mul(rI3[:], ri3, rcol[:])
            negmr = spool.tile([3, 1], F32, tag="negmr")
            nc.vector.tensor_scalar_mul(negmr[:], negmean[:], rcol[:])
            nm96ps = psm.tile([1, 96], F32, tag="sm")
            nc.tensor.matmul(nm96ps[:], negmr[:], i3t, start=True, stop=True)
            negmr96 = spool.tile([1, 96], F32, tag="negmr96")
            nc.vector.tensor_copy(negmr96[:], nm96ps[:])

            # point-major output: fin_pm[p, 3b:3b+3] = r*opre.T + bias
            finpm_ps = psm.tile([128, 96], F32, tag="sm")
            fin = cl1pool.tile([128, 96], F32, tag="fin")
            for g in range(8):
                gsl = slice(12 * g, 12 * (g + 1))
                nc.tensor.matmul(finpm_ps[:, gsl], ones1,
                                 negmr96[:, gsl], start=True, stop=False,
                                 skip_group_check=True)
                for bi in range(4):
                    b = 4 * g + bi
                    nc.tensor.matmul(
                        finpm_ps[:, 3 * b:3 * (b + 1)],
                        opre[:, 128 * b:128 * (b + 1)], rI3[:],
                        start=False, stop=(bi == 3), skip_group_check=True)
                nc.vector.tensor_copy(fin[:, gsl], finpm_ps[:, gsl])
            nc.sync.dma_start(
                out_d[n].rearrange("(b p) c -> p b c", p=128),
                fin[:, :].rearrange("p (b c) -> p b c", c=3))

    nc.compile()
    return nc


_prog = None


def _get_prog():
    global _prog
    if _prog is None:
        _prog = _build_program()
    return _prog


def _spatial_grid(k):
    xs = np.linspace(-1.0, 1.0, k, dtype=np.float32)
    gx, gy = np.meshgrid(xs, xs, indexing="ij")
    return np.stack([gx.ravel(), gy.ravel()], axis=-1)


def _host_prep(inputs):
    f32 = np.float32
    sw1 = np.asarray(inputs["sw1"], f32)
    sb1 = np.asarray(inputs["sb1"], f32)
    sw2 = np.asarray(inputs["sw2"], f32)
    sb2 = np.asarray(inputs["sb2"], f32)
    gw1 = np.asarray(inputs["gw1"], f32)
    gb1 = np.asarray(inputs["gb1"], f32)
    gw2 = np.asarray(inputs["gw2"], f32)
    gb2 = np.asarray(inputs["gb2"], f32)
    fw1 = np.asarray(inputs["fw1"], f32)
    fb1 = np.asarray(inputs["fb1"], f32)
    fw2 = np.asarray(inputs["fw2"], f32)
    fb2 = np.asarray(inputs["fb2"], f32)
    fw3 = np.asarray(inputs["fw3"], f32)
    fb3 = np.asarray(inputs["fb3"], f32)

    w1aug = np.zeros((17, 384), f32)
    w1aug[0:15, 0:128] = sw1
    w1aug[0:15, 128:256] = gw1[0:15]
    w1aug[0:15, 256:384] = fw1[0:15]
    w1aug[15:17, 256:384] = fw1[527:529]

    expb = np.zeros((128, 7), f32)
    eb = np.exp(sb2).astype(f32)
    for j in range(7):
        q = QT[j]
        expb[0:q, j] = eb[128 * j:128 * j + q]

    gw1g = np.concatenate(
        [gw1[15 + 128 * j:15 + 128 * (j + 1)] for j in range(4)], axis=1)
    fw1g = np.concatenate(
        [fw1[15 + 128 * j:15 + 128 * (j + 1)] for j in range(4)], axis=1)

    vals = {
        "w1aug": w1aug,
        "sw2": sw2,
        "expb": expb,
        "expbf": expb,
        "gw1g": gw1g,
        "fw1g": fw1g,
        "fw1s": fw1[529:657],
        "gw2": gw2,
        "gb2h": (0.5 * gb2).reshape(128, 1),
        "fw2": fw2,
        "fb2": fb2.reshape(128, 1),
        "fw3": fw3,
        "fb3": fb3.reshape(3, 1),
        "sb1": sb1.reshape(128, 1),
        "gb1": gb1.reshape(128, 1),
        "fb1": fb1.reshape(128, 1),
        "coordsT": _spatial_grid(K_GRID).T,
        "ident": np.eye(128, dtype=f32),
        "bc05": np.full((1, 128), 0.5, f32),
        "ones3": np.ones((3, 1), f32),
        "ones1": np.ones((1, 128), f32),
        "ones13": np.ones((1, 3), f32),
        "i3t": np.tile(np.eye(3, dtype=f32), (1, 32)),
        "ri3": np.eye(3, dtype=f32),
    }

    def pack(specs, layout, ncols, dtype):
        blob = np.zeros((128, ncols), dtype)
        for name, rows, cols in specs:
            r, c0, cc = layout[name]
            blob[0:r, c0:c0 + cc] = np.asarray(vals[name], f32).astype(dtype)
        return blob

    return {
        "bfb": pack(_BF_SPECS, BF_L, BF_COLS, BF),
        "f32b": pack(_F32_SPECS, F32_L, F32_COLS, f32),
    }


def _in_maps(inputs):
    f32 = np.float32
    pts = np.asarray(inputs["points"], f32)
    xf = np.asarray(inputs["transform"], f32)
    filt = np.asarray(inputs["enc_filters"], f32).reshape(N, C, HW)
    glob = np.asarray(inputs["enc_glob"], f32)
    consts = _host_prep(inputs)
    maps = []
    for i in range(NCORES):
        s = slice(CPC * i, CPC * (i + 1))
        m = {
            "pts": np.ascontiguousarray(pts[s]),
            "xf": np.ascontiguousarray(xf[s]),
            "filt": np.ascontiguousarray(filt[s]),
            "glob": np.ascontiguousarray(glob[s]),
        }
        m.update(consts)
        maps.append(m)
    return maps


def run(inputs, trace=False):
    nc = _get_prog()
    maps = _in_maps(inputs)
    res = bass_utils.run_bass_kernel_spmd(
        nc, maps, core_ids=list(range(NCORES)), trace=trace)
    out = np.concatenate([res.results[i]["out"] for i in range(NCORES)],
                         axis=0)
    return out.astype(np.float32), res


def kernel(**inputs):
    out, _ = run(inputs, trace=False)
    return out


# revision 29
# speedup vs baseline: 1.3083x; 1.0101x over previous
"""Trainium2 Bass kernel for nn_AttentionFold (self-contained).

Data-parallel over batch N=16: core i processes clouds {2i, 2i+1}.
Feature-major layout on-chip: activations stored [feature, point].

Key restructurings vs the reference:
  - glob (512-dim) contribution to gate/fold hidden layers is a per-cloud
    constant -> one matvec per cloud, applied as relu bias.
  - softmax bias sb2 folded into the filters: E = exp(z), F' = F * exp(sb2),
    sumexp = exp(sb2) . E  (softmax invariant under this refactoring).
  - sigmoid(x) = 0.5 + 0.5*tanh(x/2) so one ACT table set serves the whole
    kernel (exp_and_others: exp/tanh/relu/identity/square).
  - coords grid is input-independent -> host constant, pre-transposed.
  - normalization scale 1/sqrt(max||c||^2) via DVE Newton rsqrt.
  - matmuls in bf16 (fp32 PSUM accumulate) so weight loads use FWL and hide
    behind the matmul stream; the points passthrough to the output is fp32.
  - all weight constants packed into two SBUF blobs (2 DMAs at startup).
  - output produced point-major on the PE (normalize affine folded into the
    transposing matmuls) so the store DMA uses 12-byte-run descriptors.
"""

from contextlib import ExitStack

import numpy as np
import ml_dtypes

import concourse.bass as bass
import concourse.tile as tile
from concourse import bacc, mybir
from concourse import bass_utils

F32 = mybir.dt.float32
BF16 = mybir.dt.bfloat16
U32 = mybir.dt.uint32
AF = mybir.ActivationFunctionType
ALU = mybir.AluOpType
BF = ml_dtypes.bfloat16

N, P, C, HW, G = 16, 4096, 128, 784, 512
NCORES = 8
CPC = N // NCORES          # clouds per core = 2
CH = 512                   # points per chunk
NPAIR = P // (2 * CH)      # chunk pairs per cloud = 4
QT = [128, 128, 128, 128, 128, 128, 16]   # q tiles of HW=784
K_GRID = 64

# const blob layouts: name -> (rows, col0, cols)
_BF_SPECS = [("w1aug", 17, 384), ("sw2", 128, HW), ("expb", 128, 7),
             ("fw1s", 128, 128), ("gw2", 128, 128), ("fw2", 128, 128),
             ("fw3", 128, 3), ("bc05", 1, 128), ("coordsT", 2, P)]
_F32_SPECS = [("expbf", 128, 7), ("gw1g", 128, 512), ("fw1g", 128, 512),
              ("ident", 128, 128), ("gb2h", 128, 1), ("fb2", 128, 1),
              ("fb3", 3, 1), ("sb1", 128, 1), ("gb1", 128, 1),
              ("fb1", 128, 1), ("ones3", 3, 1), ("ones1", 1, 128),
              ("ones13", 1, 3), ("i3t", 3, 96), ("ri3", 3, 3)]


def _layout(specs):
    out, c = {}, 0
    for name, rows, cols in specs:
        out[name] = (rows, c, cols)
        c += cols
    return out, c


BF_L, BF_COLS = _layout(_BF_SPECS)
F32_L, F32_COLS = _layout(_F32_SPECS)


def _build_program():
    nc = bacc.Bacc("TRN2", target_bir_lowering=False, debug=False,
                   num_devices=NCORES)

    def din(name, shape, dt=F32):
        return nc.dram_tensor(name, shape, dt, kind="ExternalInput").ap()

    pts_d = din("pts", (CPC, P, 3))
    xf_d = din("xf", (CPC, P, 12))
    filt_d = din("filt", (CPC, C, HW))
    glob_d = din("glob", (CPC, G))
    bfb_d = din("bfb", (128, BF_COLS), BF16)
    f32b_d = din("f32b", (128, F32_COLS))
    out_d = nc.dram_tensor("out", (CPC, P, 3), F32, kind="ExternalOutput").ap()

    with tile.TileContext(nc) as tc, ExitStack() as ctx:
        cpool = ctx.enter_context(tc.tile_pool(name="consts", bufs=1))
        clpool = ctx.enter_context(tc.tile_pool(name="cloud", bufs=2))
        cl1pool = ctx.enter_context(tc.tile_pool(name="cloud1", bufs=1))
        spool = ctx.enter_context(tc.tile_pool(name="acts", bufs=2))
        epool = ctx.enter_context(tc.tile_pool(name="e", bufs=2))
        # PSUM: 8 banks = z pair-wide (2 x 1) + w (1 x 4) + sm (1 x 2)
        pz = ctx.enter_context(tc.tile_pool(name="pz", bufs=1, space="PSUM"))
        pw = ctx.enter_context(tc.tile_pool(name="pw", bufs=4, space="PSUM"))
        psm = ctx.enter_context(tc.tile_pool(name="psm", bufs=2, space="PSUM"))

        bfb = cpool.tile([128, BF_COLS], BF16, tag="bfb")
        nc.sync.dma_start(bfb[:], bfb_d[:])
        f32b = cpool.tile([128, F32_COLS], F32, tag="f32b")
        nc.sync.dma_start(f32b[:], f32b_d[:])

        def bfc(name):
            r, c0, cc = BF_L[name]
            return bfb[0:r, c0:c0 + cc]

        def f32c(name):
            r, c0, cc = F32_L[name]
            return f32b[0:r, c0:c0 + cc]

        w1aug, sw2, expb = bfc("w1aug"), bfc("sw2"), bfc("expb")
        fw1s, gw2, fw2, fw3 = bfc("fw1s"), bfc("gw2"), bfc("fw2"), bfc("fw3")
        bc05, coordsT = bfc("bc05"), bfc("coordsT")
        expbf, gw1g, fw1g, ident = (f32c("expbf"), f32c("gw1g"),
                                    f32c("fw1g"), f32c("ident"))
        gb2h, fb2, fb3, sb1, gb1, fb1 = (f32c("gb2h"), f32c("fb2"),
                                         f32c("fb3"), f32c("sb1"),
                                         f32c("gb1"), f32c("fb1"))
        ones3, ones1, i3t, ri3 = (f32c("ones3"), f32c("ones1"),
                                  f32c("i3t"), f32c("ri3"))
        ones13 = f32c("ones13")
        rsqC = cpool.tile([1, 1], U32, tag="rsqC")
        nc.vector.memset(rsqC[:], 0x5F3759DF)

        for n in range(CPC):
            # ---- per-cloud prep ----
            Fsb = clpool.tile([128, HW], F32, tag="Fsb")
            nc.sync.dma_start(Fsb[:], filt_d[n])
            glob_sb = clpool.tile([128, 4], F32, tag="glob")
            nc.sync.dma_start(
                glob_sb[:], glob_d[n].rearrange("(c p) -> p c", p=128))

            # F' = (F * exp(sb2)) transposed -> FT7 [q, c] tiles (bf16)
            FT7 = clpool.tile([128, 896], BF16, tag="FT7")
            for j in range(7):
                q = QT[j]
                ftp = pw.tile([128, 128], F32, tag="w")
                nc.tensor.transpose(
                    ftp[0:q, :], Fsb[:, 128 * j:128 * j + q], ident)
                nc.vector.tensor_scalar_mul(
                    FT7[0:q, 128 * j:128 * (j + 1)], ftp[0:q, :],
                    expbf[0:q, j:j + 1])

            # glob matvecs -> per-cloud gate/fold relu biases
            gps = pw.tile([128, 1], F32, tag="w")
            for j in range(4):
                nc.tensor.matmul(
                    gps[:], gw1g[:, 128 * j:128 * (j + 1)],
                    glob_sb[:, j:j + 1], start=(j == 0), stop=(j == 3))
            gbias = clpool.tile([128, 1], F32, tag="gbias")
            nc.vector.tensor_tensor(gbias[:], gps[:], gb1, ALU.add)
            fps = pw.tile([128, 1], F32, tag="w")
            for j in range(4):
                nc.tensor.matmul(
                    fps[:], fw1g[:, 128 * j:128 * (j + 1)],
                    glob_sb[:, j:j + 1], start=(j == 0), stop=(j == 3))
            fbias = clpool.tile([128, 1], F32, tag="fbias")
            nc.vector.tensor_tensor(fbias[:], fps[:], fb1, ALU.add)

            opre = clpool.tile([3, P], F32, tag="opre")
            msum = clpool.tile([3, 2 * NPAIR], F32, tag="msum")

            # point-major staging for pts+transform -> feature-major via PE
            Xpm = clpool.tile([128, 480], F32, tag="Xpm")
            Xpm3 = Xpm[:, :].rearrange("p (b k) -> p b k", k=15)
            nc.gpsimd.dma_start(
                Xpm3[:, :, 0:3], pts_d[n].rearrange("(b p) k -> p b k", p=128))
            nc.gpsimd.dma_start(
                Xpm3[:, :, 3:15], xf_d[n].rearrange("(b p) k -> p b k", p=128))

            X17c = clpool.tile([17, P], BF16, tag="X17c")
            pts32c = clpool.tile([3, P], F32, tag="pts32c")
            for c in range(2 * NPAIR):
                XT = pw.tile([15, CH], F32, tag="w")
                for b in range(4):
                    nc.tensor.transpose(
                        XT[:, 128 * b:128 * (b + 1)],
                        Xpm[:, 15 * (4 * c + b):15 * (4 * c + b) + 15],
                        ident)
                sl = slice(CH * c, CH * (c + 1))
                nc.vector.tensor_copy(X17c[0:15, sl], XT[:])
                nc.scalar.copy(pts32c[:, sl], XT[0:3, :])
            nc.sync.dma_start(X17c[15:17, :], coordsT)

            # ---- chunk pairs ----
            for cp in range(NPAIR):
                sh2 = spool.tile([128, 2 * CH], BF16, tag="sh2")
                ghs = []
                for h in range(2):
                    c = 2 * cp + h
                    sl = slice(CH * c, CH * (c + 1))
                    sh_ps = pw.tile([128, CH], F32, tag="w")
                    nc.tensor.matmul(sh_ps[:], w1aug[:, 0:128], X17c[:, sl],
                                     start=True, stop=True)
                    gh_ps = pw.tile([128, CH], F32, tag="w")
                    nc.tensor.matmul(gh_ps[:], w1aug[:, 128:256], X17c[:, sl],
                                     start=True, stop=True)
                    nc.vector.tensor_scalar(
                        sh2[:, CH * h:CH * (h + 1)], sh_ps[:], sb1, 0.0,
                        ALU.add, ALU.max)
                    gh = spool.tile([128, CH], BF16, tag=f"gh_{h}")
                    nc.vector.tensor_scalar(gh[:], gh_ps[:], gbias[:],
                                            0.0, ALU.add, ALU.max)
                    ghs.append(gh)

                # gate second layer early (PE filler during exp rounds)
                gts = []
                for h in range(2):
                    ga = pw.tile([128, CH], F32, tag="w")
                    nc.tensor.matmul(ga[:], gw2, ghs[h][:],
                                     start=True, stop=True)
                    gt = spool.tile([128, CH], F32, tag=f"gt_{h}")
                    nc.scalar.activation(gt[:], ga[:], AF.Tanh,
                                         bias=gb2h, scale=0.5)
                    gts.append(gt)

                # z/exp rounds interleaved with the previous round's attention
                E = epool.tile([128, 7 * 2 * CH], BF16, tag="E")
                spat0 = pw.tile([128, CH], F32, tag="w")
                spat1 = pw.tile([128, CH], F32, tag="w")
                sume0 = psm.tile([1, CH], F32, tag="sm")
                sume1 = psm.tile([1, CH], F32, tag="sm")
                spats = [spat0, spat1]
                sumes = [sume0, sume1]

                def attn_round(j):
                    q = QT[j]
                    for h in range(2):
                        esl = E[0:q, 1024 * j + CH * h:1024 * j + CH * (h + 1)]
                        nc.tensor.matmul(
                            spats[h][:], FT7[0:q, 128 * j:128 * (j + 1)], esl,
                            start=(j == 0), stop=(j == 6))
                        nc.tensor.matmul(
                            sumes[h][:], expb[0:q, j:j + 1], esl,
                            start=(j == 0), stop=(j == 6))

                for j in range(7):
                    q = QT[j]
                    zt = pz.tile([128, 2 * CH], F32, tag="z")
                    for h in range(2):
                        nc.tensor.matmul(zt[0:q, CH * h:CH * (h + 1)],
                                         sw2[:, 128 * j:128 * j + q],
                                         sh2[:, CH * h:CH * (h + 1)],
                                         start=True, stop=True)
                    nc.scalar.activation(
                        E[0:q, 1024 * j:1024 * (j + 1)], zt[0:q, :], AF.Exp)
                    if j > 0:
                        attn_round(j - 1)
                attn_round(6)

                g1s = []
                for h in range(2):
                    g1 = spool.tile([128, CH], F32, tag=f"g1_{h}")
                    nc.vector.scalar_tensor_tensor(
                        g1[:], gts[h][:], 1.0, spats[h][:], ALU.add, ALU.mult)
                    g1s.append(g1)

                for h in range(2):
                    c = 2 * cp + h
                    sl = slice(CH * c, CH * (c + 1))
                    sume, g1 = sumes[h], g1s[h]

                    rinv = spool.tile([1, CH], BF16, tag="rinv")
                    with nc.allow_low_precision(reason="bf16 softmax scale"):
                        nc.vector.reciprocal(rinv[:], sume[:])
                    rbc = psm.tile([128, CH], F32, tag="sm")
                    nc.tensor.matmul(rbc[:], bc05, rinv[:],
                                     start=True, stop=True)
                    feats = spool.tile([128, CH], BF16, tag="feats")
                    nc.vector.tensor_tensor(feats[:], g1[:], rbc[:], ALU.mult)

                    f1ps = pw.tile([128, CH], F32, tag="w")
                    nc.tensor.matmul(f1ps[:], w1aug[:, 256:384], X17c[:, sl],
                                     start=True, stop=False)
                    nc.tensor.matmul(f1ps[:], fw1s, feats[:],
                                     start=False, stop=True)
                    f1 = spool.tile([128, CH], BF16, tag="f1s")
                    nc.vector.tensor_scalar(f1[:], f1ps[:], fbias[:], 0.0,
                                            ALU.add, ALU.max)
                    f2ps = pw.tile([128, CH], F32, tag="w")
                    nc.tensor.matmul(f2ps[:], fw2, f1[:],
                                     start=True, stop=True)
                    f2 = spool.tile([128, CH], BF16, tag="f2s")
                    nc.vector.tensor_scalar(f2[:], f2ps[:], fb2, 0.0,
                                            ALU.add, ALU.max)
                    f3ps = psm.tile([3, CH], F32, tag="sm")
                    nc.tensor.matmul(f3ps[:], fw3, f2[:],
                                     start=True, stop=True)
                    # opre = f3 + fb3 + points (fp32 path); accum -> mean
                    nc.vector.scalar_tensor_tensor(
                        opre[:, sl], f3ps[:], fb3, pts32c[:, sl],
                        ALU.add, ALU.add, accum_out=msum[:, c:c + 1])

            # ---- per-cloud normalize ----
            msr = spool.tile([3, 1], F32, tag="msr")
            nc.vector.reduce_sum(msr[:], msum[:], axis=mybir.AxisListType.X)
            negmean = spool.tile([3, 1], F32, tag="negmean")
            nc.vector.tensor_scalar_mul(negmean[:], msr[:], -1.0 / P)
            sqc = cl1pool.tile([3, P], F32, tag="sqc")
            nc.scalar.activation(sqc[:], opre[:], AF.Square,
                                 bias=negmean[:], scale=1.0)
            n2 = psm.tile([128, 32], F32, tag="sm")
            for c in range(2 * NPAIR):
                for b in range(4):
                    nc.tensor.matmul(
                        n2[:, 4 * c + b:4 * c + b + 1],
                        sqc[:, 512 * c + 128 * b:512 * c + 128 * (b + 1)],
                        ones3, start=True, stop=True)
            nm128 = spool.tile([128, 1], F32, tag="nm128")
            nc.vector.reduce_max(nm128[:], n2[:], axis=mybir.AxisListType.X)
            nmT = psm.tile([1, 128], F32, tag="sm")
            nc.tensor.transpose(nmT[:], nm128[:], ident)
            nmax = spool.tile([1, 1], F32, tag="nmax")
            nc.vector.reduce_max(nmax[:], nmT[:], axis=mybir.AxisListType.X)

            # Newton rsqrt: y ~= 1/sqrt(nmax)
            ysh = spool.tile([1, 1], U32, tag="ysh")
            nc.vector.tensor_scalar(ysh[:], nmax[:].bitcast(U32), 1, None,
                                    ALU.logical_shift_right)
            y = spool.tile([1, 1], F32, tag="y")
            nc.vector.tensor_tensor(y[:].bitcast(U32), rsqC[:], ysh[:],
                                    ALU.subtract)
            t = spool.tile([1, 1], F32, tag="t")
            for _ in range(4):
                nc.vector.tensor_tensor(t[:], y[:], y[:], ALU.mult)
                nc.vector.tensor_tensor(t[:], t[:], nmax[:], ALU.mult)
                nc.vector.tensor_scalar(t[:], t[:], -0.5, 1.5,
                                        ALU.mult, ALU.add)
                nc.vector.tensor_tensor(y[:], y[:], t[:], ALU.mult)

            # rcol = y broadcast to 3 partitions
            rcolps = psm.tile([3, 1], F32, tag="sm")
            nc.tensor.matmul(rcolps[:], ones13, y[:], start=True, stop=True)
            rcol = spool.tile([3, 1], F32, tag="rcol")
            nc.vector.tensor_copy(rcol[:], rcolps[:])
            # rI3 = y * I3 ; negmr96 = tiled row of (negmean * y)
            rI3 = spool.tile([3, 3], F32, tag="rI3")
            nc.vector.tensor_scalar_mul(rI3[:], ri3, rcol[:])
            negmr = spool.tile([3, 1], F32, tag="negmr")
            nc.vector.tensor_scalar_mul(negmr[:], negmean[:], rcol[:])
            nm96ps = psm.tile([1, 96], F32, tag="sm")
            nc.tensor.matmul(nm96ps[:], negmr[:], i3t, start=True, stop=True)
            negmr96 = spool.tile([1, 96], F32, tag="negmr96")
            nc.vector.tensor_copy(negmr96[:], nm96ps[:])

            # point-major output: fin_pm[p, 3b:3b+3] = r*opre.T + bias
            finpm_ps = psm.tile([128, 96], F32, tag="sm")
            fin = cl1pool.tile([128, 96], F32, tag="fin")
            for g in range(8):
                gsl = slice(12 * g, 12 * (g + 1))
                nc.tensor.matmul(finpm_ps[:, gsl], ones1,
                                 negmr96[:, gsl], start=True, stop=False,
                                 skip_group_check=True)
                for bi in range(4):
                    b = 4 * g + bi
                    nc.tensor.matmul(
                        finpm_ps[:, 3 * b:3 * (b + 1)],
                        opre[:, 128 * b:128 * (b + 1)], rI3[:],
                        start=False, stop=(bi == 3), skip_group_check=True)
                nc.vector.tensor_copy(fin[:, gsl], finpm_ps[:, gsl])
                nc.sync.dma_start(
                    out_d[n, 512 * g:512 * (g + 1), :].rearrange(
                        "(b p) c -> p b c", p=128),
                    fin[:, gsl].rearrange("p (b c) -> p b c", c=3))

    nc.compile()
    return nc


_prog = None


def _get_prog():
    global _prog
    if _prog is None:
        _prog = _build_program()
    return _prog


def _spatial_grid(k):
    xs = np.linspace(-1.0, 1.0, k, dtype=np.float32)
    gx, gy = np.meshgrid(xs, xs, indexing="ij")
    return np.stack([gx.ravel(), gy.ravel()], axis=-1)


def _host_prep(inputs):
    f32 = np.float32
    sw1 = np.asarray(inputs["sw1"], f32)
    sb1 = np.asarray(inputs["sb1"], f32)
    sw2 = np.asarray(inputs["sw2"], f32)
    sb2 = np.asarray(inputs["sb2"], f32)
    gw1 = np.asarray(inputs["gw1"], f32)
    gb1 = np.asarray(inputs["gb1"], f32)
    gw2 = np.asarray(inputs["gw2"], f32)
    gb2 = np.asarray(inputs["gb2"], f32)
    fw1 = np.asarray(inputs["fw1"], f32)
    fb1 = np.asarray(inputs["fb1"], f32)
    fw2 = np.asarray(inputs["fw2"], f32)
    fb2 = np.asarray(inputs["fb2"], f32)
    fw3 = np.asarray(inputs["fw3"], f32)
    fb3 = np.asarray(inputs["fb3"], f32)

    w1aug = np.zeros((17, 384), f32)
    w1aug[0:15, 0:128] = sw1
    w1aug[0:15, 128:256] = gw1[0:15]
    w1aug[0:15, 256:384] = fw1[0:15]
    w1aug[15:17, 256:384] = fw1[527:529]

    expb = np.zeros((128, 7), f32)
    eb = np.exp(sb2).astype(f32)
    for j in range(7):
        q = QT[j]
        expb[0:q, j] = eb[128 * j:128 * j + q]

    gw1g = np.concatenate(
        [gw1[15 + 128 * j:15 + 128 * (j + 1)] for j in range(4)], axis=1)
    fw1g = np.concatenate(
        [fw1[15 + 128 * j:15 + 128 * (j + 1)] for j in range(4)], axis=1)

    vals = {
        "w1aug": w1aug,
        "sw2": sw2,
        "expb": expb,
        "expbf": expb,
        "gw1g": gw1g,
        "fw1g": fw1g,
        "fw1s": fw1[529:657],
        "gw2": gw2,
        "gb2h": (0.5 * gb2).reshape(128, 1),
        "fw2": fw2,
        "fb2": fb2.reshape(128, 1),
        "fw3": fw3,
        "fb3": fb3.reshape(3, 1),
        "sb1": sb1.reshape(128, 1),
        "gb1": gb1.reshape(128, 1),
        "fb1": fb1.reshape(128, 1),
        "coordsT": _spatial_grid(K_GRID).T,
        "ident": np.eye(128, dtype=f32),
        "bc05": np.full((1, 128), 0.5, f32),
        "ones3": np.ones((3, 1), f32),
        "ones1": np.ones((1, 128), f32),
        "ones13": np.ones((1, 3), f32),
        "i3t": np.tile(np.eye(3, dtype=f32), (1, 32)),
        "ri3": np.eye(3, dtype=f32),
    }

    def pack(specs, layout, ncols, dtype):
        blob = np.zeros((128, ncols), dtype)
        for name, rows, cols in specs:
            r, c0, cc = layout[name]
            blob[0:r, c0:c0 + cc] = np.asarray(vals[name], f32).astype(dtype)
        return blob

    return {
        "bfb": pack(_BF_SPECS, BF_L, BF_COLS, BF),
        "f32b": pack(_F32_SPECS, F32_L, F32_COLS, f32),
    }


def _in_maps(inputs):
    f32 = np.float32
    pts = np.asarray(inputs["points"], f32)
    xf = np.asarray(inputs["transform"], f32)
    filt = np.asarray(inputs["enc_filters"], f32).reshape(N, C, HW)
    glob = np.asarray(inputs["enc_glob"], f32)
    consts = _host_prep(inputs)
    maps = []
    for i in range(NCORES):
        s = slice(CPC * i, CPC * (i + 1))
        m = {
            "pts": np.ascontiguousarray(pts[s]),
            "xf": np.ascontiguousarray(xf[s]),
            "filt": np.ascontiguousarray(filt[s]),
            "glob": np.ascontiguousarray(glob[s]),
        }
        m.update(consts)
        maps.append(m)
    return maps


def run(inputs, trace=False):
    nc = _get_prog()
    maps = _in_maps(inputs)
    res = bass_utils.run_bass_kernel_spmd(
        nc, maps, core_ids=list(range(NCORES)), trace=trace)
    out = np.concatenate([res.results[i]["out"] for i in range(NCORES)],
                         axis=0)
    return out.astype(np.float32), res


def kernel(**inputs):
    out, _ = run(inputs, trace=False)
    return out
